# revision 1
# baseline (speedup 1.0000x reference)
"""DepletionLSTM Trainium2 kernel.

Self-contained: builds a Bass/Tile kernel for the 2-layer-LSTM network,
shards the batch over 8 NeuronCores (pure data parallelism), runs via
PJRT/axon, returns the full [8192, 30] float32 output.

Strategy (per core, 1024 batch):
- Everything resident in SBUF; no DRAM round-trips for activations.
- Feature-major layout: activations are [H=128 partitions, batch] tiles.
- Input-projection LayerNorm stats are computed in a prepass directly in
  [T=90 partitions, batch] layout using the quadratic-form identity
  sum_h p_h^2 = x^T (W^T W) x + 2 (W^T b)^T x + |b|^2 (F=7 is tiny, so the
  F-contractions are unrolled on the vector engine).  rsqrt is batched into
  a single Sqrt activation so the ACT table never switches inside the loop.
- Per step: x_t is PE-transposed to feature-major and pre-scaled by rstd
  (LN scaling commutes through the projection matmul); the projection plus a
  K=2 rank-2 term (b_in*rstd and -mean*rstd rows) accumulates in PSUM and a
  single DVE copy produces the normalized LSTM input.  Each LSTM layer is 4
  accumulating gate matmul pairs (input + recurrent), 4 sigmoid/tanh ACT ops
  with the gate bias folded into the activation bias, tanh(c), and 4 DVE
  elementwise ops.  Layer 1 runs one timestep behind layer 0 (double-buffered
  h0) so both layers' engine work overlaps.
- Matmul operands use float32r (fp32 bytes, single-pass PE) for speed.

PSUM (8 banks): "pg" gates/head 2x[128,1024] (4), "pp" projection [128,1024]
(2), "pxt" x-transposes 2x[7,512] (2).
"""
import sys
sys.path.insert(0, '/opt/trn_rl_repo')

import numpy as np

B, T, F, H, D1, D2, OUT = 8192, 90, 7, 128, 128, 64, 30
NCORES = 8
BL = B // NCORES
G4 = 4 * H
NH = BL // 512
QB = BL // 128
EPS = 1e-5
MMDT = "float32r"
V_ON_POOL = False
XFMR_ON_POOL = False
PGBUFS = 2


def _build(nc, T_steps=T, mmdt_name=MMDT, dbg=False):
    global V_ON_POOL, XFMR_ON_POOL, PGBUFS
    import concourse.tile as tile
    from concourse import mybir
    from concourse.masks import make_identity

    f32 = mybir.dt.float32
    mmdt = getattr(mybir.dt, mmdt_name)
    AF = mybir.ActivationFunctionType
    ALU = mybir.AluOpType

    # ---------------- DRAM I/O ----------------
    x_d = nc.dram_tensor("x", [BL, T, F], f32, kind="ExternalInput")
    W_in_d = nc.dram_tensor("W_in", [H, F], f32, kind="ExternalInput")
    b_in_d = nc.dram_tensor("b_in", [H], f32, kind="ExternalInput")
    g_in_d = nc.dram_tensor("g_in", [H], f32, kind="ExternalInput")
    be_in_d = nc.dram_tensor("be_in", [H], f32, kind="ExternalInput")
    Wih_d = [nc.dram_tensor("Wih0", [G4, H], f32, kind="ExternalInput"),
             nc.dram_tensor("Wih1", [G4, H], f32, kind="ExternalInput")]
    Whh_d = [nc.dram_tensor("Whh0", [G4, H], f32, kind="ExternalInput"),
             nc.dram_tensor("Whh1", [G4, H], f32, kind="ExternalInput")]
    bih_d = [nc.dram_tensor("bih0", [G4], f32, kind="ExternalInput"),
             nc.dram_tensor("bih1", [G4], f32, kind="ExternalInput")]
    bhh_d = [nc.dram_tensor("bhh0", [G4], f32, kind="ExternalInput"),
             nc.dram_tensor("bhh1", [G4], f32, kind="ExternalInput")]
    g_ln_d = nc.dram_tensor("g_ln", [H], f32, kind="ExternalInput")
    be_ln_d = nc.dram_tensor("be_ln", [H], f32, kind="ExternalInput")
    W_d1_d = nc.dram_tensor("W_d1", [D1, H], f32, kind="ExternalInput")
    b_d1_d = nc.dram_tensor("b_d1", [D1], f32, kind="ExternalInput")
    W_d2_d = nc.dram_tensor("W_d2", [D2, D1], f32, kind="ExternalInput")
    b_d2_d = nc.dram_tensor("b_d2", [D2], f32, kind="ExternalInput")
    W_d3_d = nc.dram_tensor("W_d3", [OUT, D2], f32, kind="ExternalInput")
    b_d3_d = nc.dram_tensor("b_d3", [OUT], f32, kind="ExternalInput")
    out_d = nc.dram_tensor("out", [BL, OUT], f32, kind="ExternalOutput")
    if dbg:
        dbg_xfm = nc.dram_tensor("dbg_xfm", [F, BL], f32, kind="ExternalOutput")
        dbg_stats = nc.dram_tensor("dbg_stats", [2, BL], f32, kind="ExternalOutput")
        dbg_x0 = nc.dram_tensor("dbg_x0", [H, BL], f32, kind="ExternalOutput")
        dbg_h0 = nc.dram_tensor("dbg_h0", [H, BL], f32, kind="ExternalOutput")
        dbg_c0 = nc.dram_tensor("dbg_c0", [H, BL], f32, kind="ExternalOutput")
        dbg_pp = nc.dram_tensor("dbg_pp", [H, BL], f32, kind="ExternalOutput")
        dbg_rbc = nc.dram_tensor("dbg_rbc", [2, BL], f32, kind="ExternalOutput")

    import contextlib
    with tile.TileContext(nc) as tc, contextlib.ExitStack() as ctx:
        singles = ctx.enter_context(tc.tile_pool(name="singles", bufs=1))
        trans = ctx.enter_context(tc.tile_pool(name="trans", bufs=2))
        small = ctx.enter_context(tc.tile_pool(name="small", bufs=2))
        ps_pg = ctx.enter_context(tc.tile_pool(name="ps_pg", bufs=PGBUFS, space="PSUM"))
        ps_pp = ctx.enter_context(tc.tile_pool(name="ps_pp", bufs=1, space="PSUM"))
        ps_px = ctx.enter_context(tc.tile_pool(name="ps_px", bufs=2, space="PSUM"))
        dpool = ctx.enter_context(tc.tile_pool(name="dpool", bufs=1, space="DRAM"))

        def pg_tile(shape, name):
            return ps_pg.tile(shape, f32, tag="pg", name=name)

        def pp_tile(shape, name):
            return ps_pp.tile(shape, f32, tag="pp", name=name)

        def px_tile(shape, name):
            return ps_px.tile(shape, f32, tag="pxt", name=name)

        def R(ap):
            return ap

        # ---------------- constants ----------------
        ident = singles.tile([128, 128], f32)
        make_identity(nc, ident)
        ones_row = singles.tile([1, 512], f32)
        nc.vector.memset(ones_row, 1.0)
        ones_col = singles.tile([128, 1], f32)
        nc.vector.memset(ones_col, 1.0)
        eps_col = singles.tile([T, 1], f32)
        nc.vector.memset(eps_col, EPS)

        def load_col(dram_vec, n, name):
            t_ = singles.tile([n, 1], f32, name=name, tag=name)
            nc.sync.dma_start(out=t_, in_=dram_vec[:].rearrange("(p o) -> p o", o=1))
            return t_

        g_in_c = load_col(g_in_d, H, "g_in_c")
        be_in_c = load_col(be_in_d, H, "be_in_c")
        b_in_c = load_col(b_in_d, H, "b_in_c")
        g_ln_c = load_col(g_ln_d, H, "g_ln_c")
        be_ln_c = load_col(be_ln_d, H, "be_ln_c")
        b_d1_c = load_col(b_d1_d, D1, "b_d1_c")
        b_d2_c = load_col(b_d2_d, D2, "b_d2_c")
        b_d3_c = load_col(b_d3_d, OUT, "b_d3_c")
        b_in_row = singles.tile([1, H], f32)
        nc.sync.dma_start(out=b_in_row, in_=b_in_d[:].rearrange("(o p) -> o p", o=1))
        bn1_dram = dpool.tile([2, H], f32)
        nc.sync.dma_start(out=bn1_dram[0:1, :],
                          in_=b_in_d[:].rearrange("(o p) -> o p", o=1))
        nc.sync.dma_start(out=bn1_dram[1:2, :], in_=ones_row[:, 0:H])
        bn1 = singles.tile([2, H], f32)
        nc.sync.dma_start(out=bn1, in_=bn1_dram[:, :])

        # ---------------- weights: load + PE-transpose ----------------
        def transpose_to(dst, src_ap, p, fdim):
            pt = pp_tile([fdim, p], "tr_ps")
            nc.tensor.transpose(pt, src_ap, ident[:p, :p])
            nc.vector.tensor_copy(out=dst, in_=pt)

        w_in_raw = singles.tile([H, F], f32)
        nc.sync.dma_start(out=w_in_raw, in_=W_in_d[:, :])
        w_inT = singles.tile([F, H], mmdt)
        transpose_to(w_inT, w_in_raw, H, F)

        wihT0f = singles.tile([H, 4, H], f32)
        wihT, whhT = [], []
        for L in range(2):
            wt = singles.tile([H, 4, H], mmdt, name=f"wihT{L}", tag=f"wihT{L}")
            ht = singles.tile([H, 4, H], mmdt, name=f"whhT{L}", tag=f"whhT{L}")
            for cc in range(4):
                raw = trans.tile([H, H], f32, tag="u", name="raw")
                nc.sync.dma_start(out=raw, in_=Wih_d[L][cc * H:(cc + 1) * H, :])
                pt_w = pp_tile([H, H], "tr_ps_w")
                nc.tensor.transpose(pt_w, raw, ident)
                nc.vector.tensor_copy(out=wt[:, cc, :], in_=pt_w)
                if L == 0:
                    nc.vector.tensor_copy(out=wihT0f[:, cc, :], in_=pt_w)
                raw2 = trans.tile([H, H], f32, tag="v_", name="raw2")
                nc.sync.dma_start(out=raw2, in_=Whh_d[L][cc * H:(cc + 1) * H, :])
                transpose_to(ht[:, cc, :], raw2, H, H)
            wihT.append(wt)
            whhT.append(ht)

        # gate biases beff[L] [128, 4]; layer-0 gains Wih0 @ be_in (beta fold)
        beff = []
        for L in range(2):
            bt_ = singles.tile([H, 4], f32, name=f"beff{L}", tag=f"beff{L}")
            bih_sb = small.tile([H, 4], f32, tag="bload", name="bih_sb")
            nc.sync.dma_start(out=bih_sb,
                              in_=bih_d[L][:].rearrange("(c p) -> p c", p=H))
            bhh_sb = small.tile([H, 4], f32, tag="bload2", name="bhh_sb")
            nc.sync.dma_start(out=bhh_sb,
                              in_=bhh_d[L][:].rearrange("(c p) -> p c", p=H))
            nc.vector.tensor_add(out=bt_, in0=bih_sb, in1=bhh_sb)
            beff.append(bt_)
        for cc in range(4):
            pb = px_tile([H, 1], "pb")
            nc.tensor.matmul(pb, wihT0f[:, cc, :], be_in_c, start=True, stop=True)
            nc.vector.tensor_add(out=beff[0][:, cc:cc + 1],
                                 in0=beff[0][:, cc:cc + 1], in1=pb)
        # gamma-fold layer-0 input weights (rows scaled by g_in)
        nc.vector.tensor_scalar_mul(
            out=wihT[0][:, :, :].rearrange("p c m -> p (c m)"),
            in0=wihT[0][:, :, :].rearrange("p c m -> p (c m)"),
            scalar1=g_in_c)

        wd1T = singles.tile([H, D1], f32)
        wd1_raw = trans.tile([D1, H], f32, tag="u", name="wd1_raw")
        nc.sync.dma_start(out=wd1_raw, in_=W_d1_d[:, :])
        transpose_to(wd1T, wd1_raw, D1, H)
        wd2T = singles.tile([D1, D2], f32)
        wd2_raw = trans.tile([D2, D1], f32, tag="v_", name="wd2_raw")
        nc.sync.dma_start(out=wd2_raw, in_=W_d2_d[:, :])
        transpose_to(wd2T, wd2_raw, D2, D1)
        wd3T = singles.tile([D2, OUT], f32)
        wd3_raw = trans.tile([OUT, D2], f32, tag="u", name="wd3_raw")
        nc.sync.dma_start(out=wd3_raw, in_=W_d3_d[:, :])
        transpose_to(wd3T, wd3_raw, OUT, D2)

        # ---------------- x loads ----------------
        # loop layout: xrow[p, t, q, f] = x[128q+p, t, f]
        xrow_all = singles.tile([128, T, QB, F], f32)
        nc.sync.dma_start(
            out=xrow_all,
            in_=x_d[:, :, :].rearrange("(q p) t f -> p t q f", p=128))
        # prepass layout: x_tm[t, q, p, f] = x[128q+p, t, f]
        x_tm = singles.tile([T, QB, 128, F], f32)
        nc.sync.dma_start(
            out=x_tm,
            in_=x_d[:, :, :].rearrange("(q p) t f -> t q p f", p=128))

        # ---------------- prepass: LN stats in [T, BL] layout ----------------
        # p' = W_in x + b_in per (h | b,t); over h:
        #   sum p'   = wsum . x + bsum
        #   sum p'^2 = x^T M x + 2 l^T x + c0,  M = W^T W, l = W^T b, c0=|b|^2
        p_m = pp_tile([F, F], "stat_m")
        nc.tensor.matmul(p_m, w_in_raw, w_in_raw, start=True, stop=True)
        p_ws = px_tile([1, F], "stat_ws")
        nc.tensor.matmul(p_ws, ones_col, w_in_raw, start=True, stop=True)
        p_l = px_tile([1, F], "stat_l")
        nc.tensor.matmul(p_l, b_in_c, w_in_raw, start=True, stop=True)
        p_sc = px_tile([1, 2], "stat_sc")
        nc.tensor.matmul(p_sc[:, 0:1], b_in_c, b_in_c, start=True, stop=False,
                         skip_group_check=True)
        nc.tensor.matmul(p_sc[:, 1:2], ones_col, b_in_c, start=False, stop=True,
                         skip_group_check=True)
        m_sb = small.tile([F, F], f32, tag="m_sb", name="m_sb")
        nc.vector.tensor_copy(out=m_sb, in_=p_m)
        ws_sb = small.tile([1, F], f32, tag="ws_sb", name="ws_sb")
        nc.vector.tensor_copy(out=ws_sb, in_=p_ws)
        l_sb = small.tile([1, F], f32, tag="l_sb", name="l_sb")
        nc.vector.tensor_copy(out=l_sb, in_=p_l)
        sc_sb = small.tile([1, 2], f32, tag="sc_sb", name="sc_sb")
        nc.vector.tensor_copy(out=sc_sb, in_=p_sc)
        # stage stat constants to DRAM, then partition-broadcast them back
        stat_dram = dpool.tile([F + 2, F * F], f32)
        nc.sync.dma_start(out=stat_dram[0:1, :].rearrange("o (a b) -> (o a) b", a=F),
                          in_=m_sb)
        nc.sync.dma_start(out=stat_dram[F:F + 1, 0:F], in_=ws_sb)
        nc.sync.dma_start(out=stat_dram[F:F + 1, F:2 * F], in_=l_sb)
        nc.sync.dma_start(out=stat_dram[F + 1:F + 2, 0:2], in_=sc_sb)
        wbc = singles.tile([T, F], f32)
        nc.gpsimd.dma_start(out=wbc, in_=stat_dram[F:F + 1, 0:F].to_broadcast([T, F]))
        lbc = singles.tile([T, F], f32)
        nc.gpsimd.dma_start(out=lbc,
                            in_=stat_dram[F:F + 1, F:2 * F].to_broadcast([T, F]))
        mbc = singles.tile([T, F * F], f32)
        nc.gpsimd.dma_start(out=mbc, in_=stat_dram[0:1, :].to_broadcast([T, F * F]))
        scbc = singles.tile([T, 2], f32)
        nc.gpsimd.dma_start(out=scbc,
                            in_=stat_dram[F + 1:F + 2, 0:2].to_broadcast([T, 2]))

        def xf(fi):
            return x_tm[:T_steps, :, :, fi].rearrange("t q p -> t (q p)")

        TS = T_steps
        nmu_all = singles.tile([T, BL], f32)
        r_all = singles.tile([T, BL], f32)
        acc = trans.tile([T, BL], f32, tag="sig_i", name="st_acc")
        nc.vector.tensor_scalar_mul(out=acc[:TS], in0=xf(0), scalar1=wbc[:TS, 0:1])
        for fi in range(1, F):
            nc.vector.scalar_tensor_tensor(
                out=acc[:TS], in0=xf(fi), scalar=wbc[:TS, fi:fi + 1],
                in1=acc[:TS], op0=ALU.mult, op1=ALU.add)
        # nmu = -(acc + bsum)/H
        nc.vector.tensor_scalar(out=nmu_all[:TS], in0=acc[:TS],
                                scalar1=scbc[:TS, 1:2], scalar2=-1.0 / H,
                                op0=ALU.add, op1=ALU.mult)
        # quadratic form
        qacc = trans.tile([T, BL], f32, tag="sig_f", name="st_qacc")
        yf = trans.tile([T, BL], f32, tag="tg", name="st_yf")
        tmp = trans.tile([T, BL], f32, tag="sig_o", name="st_tmp")
        yf2 = trans.tile([T, BL], f32, tag="sig_o", name="st_yf2")
        qacc2 = trans.tile([T, BL], f32, tag="u", name="st_qacc2")
        tmp2 = trans.tile([T, BL], f32, tag="v_", name="st_tmp2")
        for fi in range(F):
            eng = nc.vector
            y_, q_, t_ = (yf, qacc, tmp) if eng is nc.vector else (yf2, qacc2, tmp2)
            eng.tensor_scalar_mul(out=y_[:TS], in0=xf(0),
                                  scalar1=mbc[:TS, fi * F:fi * F + 1])
            for fj in range(1, F):
                eng.scalar_tensor_tensor(
                    out=y_[:TS], in0=xf(fj),
                    scalar=mbc[:TS, fi * F + fj:fi * F + fj + 1],
                    in1=y_[:TS], op0=ALU.mult, op1=ALU.add)
            eng.tensor_tensor(out=t_[:TS], in0=xf(fi), in1=y_[:TS], op=ALU.mult)
            if fi == 0:
                nc.vector.tensor_copy(out=qacc[:TS], in_=t_[:TS])
            elif fi == 2:
                nc.vector.tensor_copy(out=qacc2[:TS], in_=t_[:TS])
            elif eng is nc.vector:
                nc.vector.tensor_add(out=qacc[:TS], in0=qacc[:TS], in1=t_[:TS])
            else:
                nc.vector.tensor_add(out=qacc2[:TS], in0=qacc2[:TS], in1=t_[:TS])
        nc.vector.tensor_add(out=qacc[:TS], in0=qacc[:TS], in1=qacc2[:TS])
        # + 2 l.x
        lin = trans.tile([T, BL], f32, tag="u", name="st_lin")
        nc.vector.tensor_scalar_mul(out=lin[:TS], in0=xf(0), scalar1=lbc[:TS, 0:1])
        for fi in range(1, F):
            nc.vector.scalar_tensor_tensor(
                out=lin[:TS], in0=xf(fi), scalar=lbc[:TS, fi:fi + 1],
                in1=lin[:TS], op0=ALU.mult, op1=ALU.add)
        nc.vector.scalar_tensor_tensor(out=qacc[:TS], in0=lin[:TS], scalar=2.0,
                                       in1=qacc[:TS], op0=ALU.mult, op1=ALU.add)
        # var = (q + c0)/H - mu^2 ; r = 1/sqrt(var+eps)
        nc.vector.tensor_scalar(out=qacc[:TS], in0=qacc[:TS],
                                scalar1=scbc[:TS, 0:1], scalar2=1.0 / H,
                                op0=ALU.add, op1=ALU.mult)
        nc.vector.tensor_tensor(out=tmp[:TS], in0=nmu_all[:TS], in1=nmu_all[:TS],
                                op=ALU.mult)
        nc.vector.tensor_sub(out=qacc[:TS], in0=qacc[:TS], in1=tmp[:TS])
        nc.scalar.activation(out=r_all[:TS], in_=qacc[:TS], func=AF.Sqrt,
                             bias=eps_col[:TS], scale=1.0)
        nc.vector.reciprocal(out=r_all[:TS], in_=r_all[:TS])
        nmr_all = singles.tile([T, BL], f32)
        nc.vector.tensor_tensor(out=nmr_all[:TS], in0=nmu_all[:TS],
                                in1=r_all[:TS], op=ALU.mult)
        rnm_dram = dpool.tile([2, T, BL], f32)
        nc.sync.dma_start(out=rnm_dram[0, :TS], in_=r_all[:TS])
        nc.sync.dma_start(out=rnm_dram[1, :TS], in_=nmr_all[:TS])
        r_dram = rnm_dram[0]

        # ---------------- states ----------------
        h1 = singles.tile([H, BL], mmdt, name="h1", tag="h1")
        c = [singles.tile([H, BL], f32, name="c0", tag="c0"),
             singles.tile([H, BL], f32, name="c1", tag="c1")]
        zinit = trans.tile([H, BL], f32, tag="x0", name="zinit")
        nc.vector.memset(zinit, 0.0)
        h0_prev = trans.tile([H, BL], mmdt, tag="h0", name="h0_init")
        nc.vector.tensor_copy(out=h0_prev, in_=zinit)
        nc.vector.tensor_copy(out=h1, in_=zinit)
        for L in range(2):
            nc.vector.memset(c[L], 0.0)

        # ---------------- main loop ----------------
        def lstm_step(L, inp, hprev, hout, hh_first):
            sig_i = trans.tile([H, BL], f32, tag="sig_i", name="sig_i")
            sig_f = trans.tile([H, BL], f32, tag="sig_f", name="sig_f")
            tg = trans.tile([H, BL], f32, tag="tg", name="tg")
            sig_o = trans.tile([H, BL], f32, tag="sig_o", name="sig_o")
            outs = [sig_i, sig_f, tg, sig_o]
            funcs = [AF.Sigmoid, AF.Sigmoid, AF.Tanh, AF.Sigmoid]
            for gc in range(4):
                pg = pg_tile([H, BL], "pg_gates")
                for hc in range(NH):
                    sl = slice(hc * 512, (hc + 1) * 512)
                    ops = [(wihT[L][:, gc, :], inp), (whhT[L][:, gc, :], hprev)]
                    if hh_first:
                        ops.reverse()
                    nc.tensor.matmul(pg[:, sl], R(ops[0][0]), R(ops[0][1][:, sl]),
                                     start=True, stop=False)
                    nc.tensor.matmul(pg[:, sl], R(ops[1][0]), R(ops[1][1][:, sl]),
                                     start=False, stop=True)
                nc.scalar.activation(out=outs[gc], in_=pg, func=funcs[gc],
                                     bias=beff[L][:, gc:gc + 1], scale=1.0)
            u = trans.tile([H, BL], f32, tag="u", name="u")
            nc.vector.tensor_tensor(out=u, in0=sig_i, in1=tg, op=ALU.mult)
            v_ = trans.tile([H, BL], f32, tag="v_", name="v_")
            (nc.gpsimd if V_ON_POOL else nc.vector).tensor_tensor(
                out=v_, in0=sig_f, in1=c[L], op=ALU.mult)
            nc.vector.tensor_add(out=c[L], in0=u, in1=v_)
            tc_ = trans.tile([H, BL], f32, tag="tc_", name="tc_")
            nc.scalar.activation(out=tc_, in_=c[L], func=AF.Tanh, scale=1.0)
            nc.vector.tensor_tensor(out=hout, in0=sig_o, in1=tc_, op=ALU.mult)

        for t in range(T_steps):
            # x_t -> feature-major [7, BL] via strided DMA (f-major gather)
            x_fm = trans.tile([F, BL], f32, tag="x_fm", name="x_fm")
            pxs = []
            for half in range(2):
                px = px_tile([F, 512], f"pxt{half}")
                for qi in range(4):
                    q = half * 4 + qi
                    nc.tensor.transpose(
                        px[:, qi * 128:(qi + 1) * 128],
                        xrow_all[:, t, q, :], ident)
                pxs.append(px)
            nc.vector.tensor_copy(out=x_fm[:, 0:512], in_=pxs[0])
            nc.vector.tensor_copy(out=x_fm[:, 512:1024], in_=pxs[1])
            # rstd rows: broadcast over 7 partitions + flat rows for rank-1s
            rbc7 = trans.tile([F, BL], f32, tag="rbc7", name="rbc7")
            nc.gpsimd.dma_start(out=rbc7,
                                in_=r_dram[t:t + 1, :].to_broadcast([F, BL]))
            rn = small.tile([2, BL], f32, tag="rn", name="rn")
            nc.gpsimd.dma_start(out=rn, in_=rnm_dram[:, t, :])
            # x_fm_r = x_fm * rstd (per column)
            x_fm_r = trans.tile([F, BL], mmdt, tag="x_fm_r", name="x_fm_r")
            (nc.gpsimd if XFMR_ON_POOL else nc.vector).tensor_tensor(
                out=x_fm_r, in0=x_fm, in1=rbc7, op=ALU.mult)
            # x0 = W_in @ x_fm_r + b_in x r_row + 1 x nmr_row  (PSUM)
            pp = pp_tile([H, BL], "pp_proj")
            for hc in range(NH):
                sl = slice(hc * 512, (hc + 1) * 512)
                nc.tensor.matmul(pp[:, sl], R(w_inT), R(x_fm_r[:, sl]),
                                 start=True, stop=False, skip_group_check=True)
                nc.tensor.matmul(pp[:, sl], bn1, rn[:, sl],
                                 start=False, stop=(hc == NH - 1),
                                 skip_group_check=True)
            x0 = trans.tile([H, BL], mmdt, tag="x0", name="x0")
            nc.vector.tensor_copy(out=x0, in_=pp)
            # layer 1 runs one step behind layer 0 (consumes h0 of step t-1)
            if t > 0:
                lstm_step(1, h0_prev, h1, h1, hh_first=True)
            h0_new = trans.tile([H, BL], mmdt, tag="h0", name="h0_new")
            lstm_step(0, x0, h0_prev, h0_new, hh_first=False)
            h0_prev = h0_new
            if dbg and t == 0:
                ppc = trans.tile([H, BL], f32, tag="tc_", name="ppc_dbg")
                nc.vector.tensor_copy(out=ppc, in_=pp)
                nc.sync.dma_start(out=dbg_pp[:, :], in_=ppc)
                nc.sync.dma_start(out=dbg_rbc[:, :], in_=rn)
                nc.sync.dma_start(out=dbg_xfm[:, :], in_=x_fm)
                nc.sync.dma_start(out=dbg_stats[0:1, :], in_=nmu_all[0:1, :])
                nc.sync.dma_start(out=dbg_stats[1:2, :], in_=r_all[0:1, :])
                nc.sync.dma_start(out=dbg_x0[:, :], in_=x0.bitcast(f32))
                nc.sync.dma_start(out=dbg_h0[:, :], in_=h0_new.bitcast(f32))
                nc.sync.dma_start(out=dbg_c0[:, :], in_=c[0])
        lstm_step(1, h0_prev, h1, h1, hh_first=True)

        # ---------------- head ----------------
        h1f = trans.tile([H, BL], f32, tag="x0", name="h1f")
        nc.vector.tensor_copy(out=h1f, in_=h1.bitcast(f32))
        sqh = trans.tile([H, BL], f32, tag="sig_f", name="sqh")
        nc.vector.tensor_tensor(out=sqh, in0=h1f, in1=h1f, op=ALU.mult)
        ps_s1 = pp_tile([1, BL], "ps_s1")
        ps_s2 = pp_tile([1, BL], "ps_s2")
        for hc in range(NH):
            sl = slice(hc * 512, (hc + 1) * 512)
            nc.tensor.matmul(ps_s1[:, sl], ones_col, h1f[:, sl],
                             start=True, stop=True, skip_group_check=True)
            nc.tensor.matmul(ps_s2[:, sl], ones_col, sqh[:, sl],
                             start=True, stop=True, skip_group_check=True)
        nmu_h = singles.tile([1, BL], f32, tag="nmu_h", name="nmu_h")
        nc.vector.tensor_scalar_mul(out=nmu_h, in0=ps_s1, scalar1=-1.0 / H)
        musq_h = singles.tile([1, BL], f32, tag="musq", name="musq_h")
        nc.vector.tensor_tensor(out=musq_h, in0=nmu_h, in1=nmu_h, op=ALU.mult)
        v_h = singles.tile([1, BL], f32, tag="v_h", name="v_h")
        nc.vector.tensor_scalar_mul(out=v_h, in0=ps_s2, scalar1=1.0 / H)
        nc.vector.tensor_sub(out=v_h, in0=v_h, in1=musq_h)
        nc.scalar.activation(out=v_h, in_=v_h, func=AF.Sqrt,
                             bias=eps_col[0:1], scale=1.0)
        nc.vector.reciprocal(out=v_h, in_=v_h)
        hstat_dram = dpool.tile([2, BL], f32)
        nc.sync.dma_start(out=hstat_dram[0:1, :], in_=nmu_h)
        nc.sync.dma_start(out=hstat_dram[1:2, :], in_=v_h)
        nmbc = trans.tile([H, BL], f32, tag="u", name="nmbc")
        nc.gpsimd.dma_start(out=nmbc, in_=hstat_dram[0:1, :].to_broadcast([H, BL]))
        rhbc = trans.tile([H, BL], f32, tag="sig_i", name="rhbc")
        nc.gpsimd.dma_start(out=rhbc, in_=hstat_dram[1:2, :].to_broadcast([H, BL]))
        t1 = trans.tile([H, BL], f32, tag="tg", name="t1")
        nc.vector.tensor_tensor(out=t1, in0=h1f, in1=nmbc, op=ALU.add)
        t2 = trans.tile([H, BL], f32, tag="sig_o", name="t2")
        nc.vector.tensor_tensor(out=t2, in0=t1, in1=rhbc, op=ALU.mult)
        last = trans.tile([H, BL], f32, tag="u", name="last")
        nc.vector.tensor_scalar(out=last, in0=t2, scalar1=g_ln_c,
                                scalar2=be_ln_c, op0=ALU.mult, op1=ALU.add)
        pd1 = pg_tile([D1, BL], "pd1")
        for hc in range(NH):
            sl = slice(hc * 512, (hc + 1) * 512)
            nc.tensor.matmul(pd1[:, sl], wd1T, last[:, sl], start=True, stop=True,
                             skip_group_check=True)
        d1 = trans.tile([D1, BL], f32, tag="v_", name="d1")
        nc.scalar.activation(out=d1, in_=pd1, func=AF.Relu, bias=b_d1_c, scale=1.0)
        pd2 = pg_tile([D2, BL], "pd2")
        for hc in range(NH):
            sl = slice(hc * 512, (hc + 1) * 512)
            nc.tensor.matmul(pd2[:, sl], wd2T, d1[:, sl], start=True, stop=True,
                             skip_group_check=True)
        d2 = trans.tile([D2, BL], f32, tag="tc_", name="d2")
        nc.scalar.activation(out=d2, in_=pd2, func=AF.Relu, bias=b_d2_c, scale=1.0)
        pd3 = pg_tile([OUT, BL], "pd3")
        for hc in range(NH):
            sl = slice(hc * 512, (hc + 1) * 512)
            nc.tensor.matmul(pd3[:, sl], wd3T, d2[:, sl], start=True, stop=True,
                             skip_group_check=True)
        o3 = trans.tile([OUT, BL], f32, tag="sig_f", name="o3")
        nc.scalar.activation(out=o3, in_=pd3, func=AF.Identity, bias=b_d3_c,
                             scale=1.0)
        outT = singles.tile([128, QB, OUT], f32)
        for q in range(QB):
            pot = px_tile([128, OUT], "pot")
            nc.tensor.transpose(pot, o3[:, q * 128:(q + 1) * 128],
                                ident[:OUT, :OUT])
            nc.vector.tensor_copy(out=outT[:, q, :], in_=pot)
        nc.sync.dma_start(
            out=out_d[:, :].rearrange("(q p) c -> p q c", p=128),
            in_=outT)
    return nc


_CACHE = {}


def _get_runner():
    if "runner" in _CACHE:
        return _CACHE["runner"]
    import jax
    from jax.sharding import Mesh, PartitionSpec
    from jax.experimental.shard_map import shard_map
    import concourse.bacc as bacc
    import concourse.mybir as mybir
    from concourse.bass2jax import install_neuronx_cc_hook, _bass_exec_p, \
        partition_id_tensor

    nc = bacc.Bacc()
    _build(nc)
    nc.compile()
    install_neuronx_cc_hook()

    partition_name = nc.partition_id_tensor.name if nc.partition_id_tensor else None
    in_names, out_names, out_avals, zero_outs = [], [], [], []
    for alloc in nc.m.functions[0].allocations:
        if not isinstance(alloc, mybir.MemoryLocationSet):
            continue
        name = alloc.memorylocations[0].name
        if alloc.kind == "ExternalInput":
            if name != partition_name:
                in_names.append(name)
        elif alloc.kind == "ExternalOutput":
            out_names.append(name)
            shape = tuple(alloc.tensor_shape)
            dtype = mybir.dt.np(alloc.dtype)
            out_avals.append(jax.core.ShapedArray(shape, dtype))
            zero_outs.append(np.zeros(shape, dtype))
    n_params = len(in_names)
    all_in_names = in_names + out_names + ([partition_name] if partition_name else [])

    def _body(*args):
        operands = list(args)
        if partition_name is not None:
            operands.append(partition_id_tensor())
        outs = _bass_exec_p.bind(
            *operands,
            out_avals=tuple(out_avals),
            in_names=tuple(all_in_names),
            out_names=tuple(out_names),
            lowering_input_output_aliases=(),
            sim_require_finite=False,
            sim_require_nnan=False,
            nc=nc,
        )
        return tuple(outs)

    devices = jax.devices()[:NCORES]
    mesh = Mesh(np.asarray(devices), ("core",))
    in_specs = (PartitionSpec("core"),) * (n_params + len(out_names))
    out_specs = (PartitionSpec("core"),) * len(out_names)
    sharded = jax.jit(
        shard_map(_body, mesh=mesh, in_specs=in_specs, out_specs=out_specs,
                  check_rep=False),
        keep_unused=True)
    _CACHE["runner"] = (sharded, in_names, out_names, zero_outs)
    return _CACHE["runner"]


def kernel(**inputs) -> np.ndarray:
    sharded, in_names, out_names, zero_outs = _get_runner()
    inp = {k: np.ascontiguousarray(np.asarray(v), dtype=np.float32)
           for k, v in inputs.items()}

    def core_val(name, ci):
        if name == "x":
            return inp["x"][ci * BL:(ci + 1) * BL]
        return inp[name]

    concat_in = [
        np.concatenate([core_val(n, ci) for ci in range(NCORES)], axis=0)
        for n in in_names
    ]
    concat_zeros = [
        np.zeros((NCORES * z.shape[0], *z.shape[1:]), z.dtype) for z in zero_outs
    ]
    import jax
    out_arrs = sharded(*concat_in, *concat_zeros)
    jax.block_until_ready(out_arrs)
    oi = out_names.index("out")
    full = np.asarray(out_arrs[oi]).reshape(B, OUT)
    return full.astype(np.float32)



# revision 23
# speedup vs baseline: 1.5089x; 1.5089x over previous
"""DepletionLSTM Trainium2 kernel (v2).

Self-contained: builds a Bass/Tile kernel for the 2-layer-LSTM network,
shards the batch over 8 NeuronCores (pure data parallelism), runs via
PJRT/axon, returns the full [8192, 30] float32 output.

v2 strategy (per core, 1024 batch), ACT-bound steady state ~10.4us/step:
- W_in is folded into the layer-0 gate weights: Weff = Wih0*diag(g_in)*W_in,
  with the LN mean/rstd entering as two extra "feature" rows (r, -mu*r) of a
  9-row augmented, pre-scaled x (xs = x*r computed once in the prepass).
  This removes the per-step input projection, LN apply, x0 copy and all
  per-step DMA broadcasts.
- x transposes to feature-major are done 10 timesteps at a time (one PE
  transpose per q-chunk per window) into PSUM; per step a single [9,1024]
  Pool copy produces the matmul rhs.
- Layer 1 lags layer 0 by 2 steps; tanh(c)/h-multiply run one slot after
  their gates, so the ACT engine starts every slot with ready work and is
  the saturated bottleneck: 10 table-ops x [128,1024] per slot.
- bf16 on the h path (h, sig_i, tanh_g, sig_o, tanh_c, u, Whh/Wih1) for DVE
  2x mode and cheap recurrent matmuls; f32 for c, sig_f, v and the x path
  (float32r matmuls).
"""
import sys
sys.path.insert(0, '/opt/trn_rl_repo')

import numpy as np

B, T, F, H, D1, D2, OUT = 8192, 90, 7, 128, 128, 64, 30
NCORES = 8
BL = B // NCORES
G4 = 4 * H
NH = BL // 512
QB = BL // 128
EPS = 1e-5
W = 10            # timesteps per transpose window
NW = T // W       # 9 windows
FA = F + 2        # augmented feature rows: 7 x*r + r + (-mu*r)


LABELS = {}


def _build(nc, T_steps=T, dbg=False):
    LABELS.clear()

    def LBL(ins, label):
        try:
            LABELS[ins.ins.name if hasattr(ins, "ins") else ins.name] = label
        except Exception:
            pass
        return ins

    import concourse.tile as tile
    from concourse import mybir
    from concourse.masks import make_identity

    f32 = mybir.dt.float32
    f32r = mybir.dt.float32r
    bf16 = mybir.dt.bfloat16
    AF = mybir.ActivationFunctionType
    ALU = mybir.AluOpType

    TS = T_steps

    # ---------------- DRAM I/O ----------------
    x_d = nc.dram_tensor("x", [BL, T, F], f32, kind="ExternalInput")
    W_in_d = nc.dram_tensor("W_in", [H, F], f32, kind="ExternalInput")
    b_in_d = nc.dram_tensor("b_in", [H], f32, kind="ExternalInput")
    g_in_d = nc.dram_tensor("g_in", [H], f32, kind="ExternalInput")
    be_in_d = nc.dram_tensor("be_in", [H], f32, kind="ExternalInput")
    Wih_d = [nc.dram_tensor("Wih0", [G4, H], f32, kind="ExternalInput"),
             nc.dram_tensor("Wih1", [G4, H], f32, kind="ExternalInput")]
    Whh_d = [nc.dram_tensor("Whh0", [G4, H], f32, kind="ExternalInput"),
             nc.dram_tensor("Whh1", [G4, H], f32, kind="ExternalInput")]
    bih_d = [nc.dram_tensor("bih0", [G4], f32, kind="ExternalInput"),
             nc.dram_tensor("bih1", [G4], f32, kind="ExternalInput")]
    bhh_d = [nc.dram_tensor("bhh0", [G4], f32, kind="ExternalInput"),
             nc.dram_tensor("bhh1", [G4], f32, kind="ExternalInput")]
    g_ln_d = nc.dram_tensor("g_ln", [H], f32, kind="ExternalInput")
    be_ln_d = nc.dram_tensor("be_ln", [H], f32, kind="ExternalInput")
    W_d1_d = nc.dram_tensor("W_d1", [D1, H], f32, kind="ExternalInput")
    b_d1_d = nc.dram_tensor("b_d1", [D1], f32, kind="ExternalInput")
    W_d2_d = nc.dram_tensor("W_d2", [D2, D1], f32, kind="ExternalInput")
    b_d2_d = nc.dram_tensor("b_d2", [D2], f32, kind="ExternalInput")
    W_d3_d = nc.dram_tensor("W_d3", [OUT, D2], f32, kind="ExternalInput")
    b_d3_d = nc.dram_tensor("b_d3", [OUT], f32, kind="ExternalInput")
    out_d = nc.dram_tensor("out", [BL, OUT], f32, kind="ExternalOutput")
    if dbg:
        dbg_h0 = nc.dram_tensor("dbg_h0", [H, BL], f32, kind="ExternalOutput")
        dbg_h1 = nc.dram_tensor("dbg_h1", [H, BL], f32, kind="ExternalOutput")
        dbg_xs = nc.dram_tensor("dbg_xs", [FA, BL], f32, kind="ExternalOutput")

    import contextlib
    with tile.TileContext(nc) as tc, contextlib.ExitStack() as ctx:
        singles = ctx.enter_context(tc.tile_pool(name="singles", bufs=1))
        big = ctx.enter_context(tc.tile_pool(name="big", bufs=1))
        trans = ctx.enter_context(tc.tile_pool(name="trans", bufs=2))
        scr = ctx.enter_context(tc.tile_pool(name="scr", bufs=1))
        ps_pg = ctx.enter_context(tc.tile_pool(name="ps_pg", bufs=3, space="PSUM"))
        ps_px = ctx.enter_context(tc.tile_pool(name="ps_px", bufs=2, space="PSUM"))
        dpool = ctx.enter_context(tc.tile_pool(name="dpool", bufs=1, space="DRAM"))

        def pg_tile(name):
            return ps_pg.tile([H, BL], f32, tag="pg", name=name)

        def px_small(shape, name):
            return ps_px.tile(shape, f32, tag="px", name=name)

        # ---------------- constants ----------------
        ident = singles.tile([128, 128], f32)
        make_identity(nc, ident)
        ones_row = singles.tile([1, 512], f32)
        nc.vector.memset(ones_row, 1.0)
        ones_col = singles.tile([128, 1], f32)
        nc.vector.memset(ones_col, 1.0)
        eps_col = singles.tile([128, 1], f32)
        nc.vector.memset(eps_col, EPS)

        def load_col(dram_vec, n, name):
            t_ = singles.tile([n, 1], f32, name=name, tag=name)
            nc.sync.dma_start(out=t_, in_=dram_vec[:].rearrange("(p o) -> p o", o=1))
            return t_

        g_in_c = load_col(g_in_d, H, "g_in_c")
        be_in_c = load_col(be_in_d, H, "be_in_c")
        b_in_c = load_col(b_in_d, H, "b_in_c")
        g_ln_c = load_col(g_ln_d, H, "g_ln_c")
        be_ln_c = load_col(be_ln_d, H, "be_ln_c")
        b_d1_c = load_col(b_d1_d, D1, "b_d1_c")
        b_d2_c = load_col(b_d2_d, D2, "b_d2_c")
        b_d3_c = load_col(b_d3_d, OUT, "b_d3_c")

        # ---------------- weights: load + PE-transpose ----------------
        def transpose_to(dst, src_ap, p, fdim):
            pt = px_small([fdim, p], "tr_ps")
            nc.tensor.transpose(pt, src_ap, ident[:p, :p])
            nc.vector.tensor_copy(out=dst, in_=pt)

        w_in_raw = singles.tile([H, F], f32)
        nc.sync.dma_start(out=w_in_raw, in_=W_in_d[:, :])

        # wihT0g: transposed Wih0 with gamma fold (f32, rhs for Weff build)
        # wihT0f: transposed Wih0 pre-gamma (for bias beta-fold)
        wihT0g = singles.tile([H, 4, H], f32)
        wihT0f = singles.tile([H, 4, H], f32)
        # bf16 recurrent weights
        wihT1 = singles.tile([H, 4, H], bf16, name="wihT1", tag="wihT1")
        whhT = [singles.tile([H, 4, H], bf16, name=f"whhT{L}", tag=f"whhT{L}")
                for L in range(2)]
        for cc in range(4):
            raw = trans.tile([H, H], f32, tag="wraw", name="raw")
            nc.sync.dma_start(out=raw, in_=Wih_d[0][cc * H:(cc + 1) * H, :])
            pt_w = px_small([H, H], "tr_ps")
            nc.tensor.transpose(pt_w, raw, ident)
            nc.vector.tensor_copy(out=wihT0f[:, cc, :], in_=pt_w)
            raw2 = trans.tile([H, H], f32, tag="wraw2", name="raw2")
            nc.sync.dma_start(out=raw2, in_=Wih_d[1][cc * H:(cc + 1) * H, :])
            transpose_to(wihT1[:, cc, :], raw2, H, H)
            for L in range(2):
                raw3 = trans.tile([H, H], f32, tag="wraw", name="raw3")
                nc.sync.dma_start(out=raw3, in_=Whh_d[L][cc * H:(cc + 1) * H, :])
                transpose_to(whhT[L][:, cc, :], raw3, H, H)
        # gamma fold: wihT0g[p, c, m] = Wih0[c*128+m, p] * g_in[p]
        nc.vector.tensor_scalar_mul(
            out=wihT0g[:, :, :].rearrange("p c m -> p (c m)"),
            in0=wihT0f[:, :, :].rearrange("p c m -> p (c m)"),
            scalar1=g_in_c)

        # gate biases beff[L] [128, 4]; layer-0 gets +Wih0 @ be_in (beta fold)
        beff = []
        for L in range(2):
            bt_ = singles.tile([H, 4], f32, name=f"beff{L}", tag=f"beff{L}")
            bih_sb = trans.tile([H, 4], f32, tag="bload", name="bih_sb")
            nc.sync.dma_start(out=bih_sb,
                              in_=bih_d[L][:].rearrange("(c p) -> p c", p=H))
            bhh_sb = trans.tile([H, 4], f32, tag="bload2", name="bhh_sb")
            nc.sync.dma_start(out=bhh_sb,
                              in_=bhh_d[L][:].rearrange("(c p) -> p c", p=H))
            nc.vector.tensor_add(out=bt_, in0=bih_sb, in1=bhh_sb)
            beff.append(bt_)
        for cc in range(4):
            pb = px_small([H, 1], "pb")
            nc.tensor.matmul(pb, wihT0f[:, cc, :], be_in_c, start=True, stop=True)
            nc.vector.tensor_add(out=beff[0][:, cc:cc + 1],
                                 in0=beff[0][:, cc:cc + 1], in1=pb)

        # weff_aug [FA=9, 4, 128] f32 (used via bitcast f32r):
        #  rows 0-6 = (Wih0*diag(g)*W_in)^T ; row 7 = Wih0@(g*b_in); row 8 = Wih0@g
        weff_aug = singles.tile([FA, 4, H], f32r)
        gb_in_c = singles.tile([H, 1], f32)
        nc.vector.tensor_tensor(out=gb_in_c, in0=b_in_c, in1=g_in_c, op=ALU.mult)
        weff_dram = dpool.tile([FA, 4 * H], f32)
        wtmp7 = trans.tile([F, 4 * H], f32, tag="wtmp7", name="wtmp7")
        wtmp1 = trans.tile([1, 4 * H], f32, tag="wtmp1", name="wtmp1")
        wtmp2 = trans.tile([1, 4 * H], f32, tag="wtmp2", name="wtmp2")
        for cc in range(4):
            pwe = px_small([F, H], "pwe")
            nc.tensor.matmul(pwe, w_in_raw, wihT0g[:, cc, :], start=True, stop=True)
            nc.vector.tensor_copy(out=wtmp7[:, cc * H:(cc + 1) * H], in_=pwe)
            pb1 = px_small([1, H], "pb1")
            nc.tensor.matmul(pb1, gb_in_c, wihT0g[:, cc, :], start=True, stop=True)
            nc.vector.tensor_copy(out=wtmp1[:, cc * H:(cc + 1) * H], in_=pb1)
            pb2 = px_small([1, H], "pb2")
            nc.tensor.matmul(pb2, g_in_c, wihT0g[:, cc, :], start=True, stop=True)
            nc.vector.tensor_copy(out=wtmp2[:, cc * H:(cc + 1) * H], in_=pb2)
        nc.sync.dma_start(out=weff_dram[0:F, :], in_=wtmp7)
        nc.sync.dma_start(out=weff_dram[F:F + 1, :], in_=wtmp1)
        nc.sync.dma_start(out=weff_dram[F + 1:F + 2, :], in_=wtmp2)
        weff_stage = trans.tile([FA, 4 * H], f32, tag="weff_stage",
                                name="weff_stage")
        nc.sync.dma_start(out=weff_stage, in_=weff_dram[:, :])
        nc.vector.tensor_copy(
            out=weff_aug[:, :, :].rearrange("p c m -> p (c m)"),
            in_=weff_stage)

        wd1T = singles.tile([H, D1], f32)
        wd1_raw = trans.tile([D1, H], f32, tag="wraw", name="wd1_raw")
        nc.sync.dma_start(out=wd1_raw, in_=W_d1_d[:, :])
        transpose_to(wd1T, wd1_raw, D1, H)
        wd2T = singles.tile([D1, D2], f32)
        wd2_raw = trans.tile([D2, D1], f32, tag="wraw2", name="wd2_raw")
        nc.sync.dma_start(out=wd2_raw, in_=W_d2_d[:, :])
        transpose_to(wd2T, wd2_raw, D2, D1)
        wd3T = singles.tile([D2, OUT], f32)
        wd3_raw = trans.tile([OUT, D2], f32, tag="wraw", name="wd3_raw")
        nc.sync.dma_start(out=wd3_raw, in_=W_d3_d[:, :])
        transpose_to(wd3T, wd3_raw, OUT, D2)

        # ---------------- prepass: LN stats in [T, BL] layout ----------------
        # p' = W_in x + b_in per (h | b,t); over h:
        #   sum p'   = wsum . x + bsum
        #   sum p'^2 = x^T M x + 2 l^T x + c0,  M = W^T W, l = W^T b, c0=|b|^2
        p_m = px_small([F, F], "stat_m")
        nc.tensor.matmul(p_m, w_in_raw, w_in_raw, start=True, stop=True)
        p_ws = px_small([1, F], "stat_ws")
        nc.tensor.matmul(p_ws, ones_col, w_in_raw, start=True, stop=True)
        p_l = px_small([1, F], "stat_l")
        nc.tensor.matmul(p_l, b_in_c, w_in_raw, start=True, stop=True)
        p_sc = px_small([1, 2], "stat_sc")
        nc.tensor.matmul(p_sc[:, 0:1], b_in_c, b_in_c, start=True, stop=False,
                         skip_group_check=True)
        nc.tensor.matmul(p_sc[:, 1:2], ones_col, b_in_c, start=False, stop=True,
                         skip_group_check=True)
        m_sb = trans.tile([F, F], f32, tag="m_sb", name="m_sb")
        nc.vector.tensor_copy(out=m_sb, in_=p_m)
        ws_sb = trans.tile([1, F], f32, tag="ws_sb", name="ws_sb")
        nc.vector.tensor_copy(out=ws_sb, in_=p_ws)
        l_sb = trans.tile([1, F], f32, tag="l_sb", name="l_sb")
        nc.vector.tensor_copy(out=l_sb, in_=p_l)
        sc_sb = trans.tile([1, 2], f32, tag="sc_sb", name="sc_sb")
        nc.vector.tensor_copy(out=sc_sb, in_=p_sc)
        # stage stat constants to DRAM, then partition-broadcast them back
        stat_dram = dpool.tile([F + 2, F * F], f32)
        nc.sync.dma_start(out=stat_dram[0:1, :].rearrange("o (a b) -> (o a) b", a=F),
                          in_=m_sb)
        nc.sync.dma_start(out=stat_dram[F:F + 1, 0:F], in_=ws_sb)
        nc.sync.dma_start(out=stat_dram[F:F + 1, F:2 * F], in_=l_sb)
        nc.sync.dma_start(out=stat_dram[F + 1:F + 2, 0:2], in_=sc_sb)
        wbc = singles.tile([128, F], f32)
        nc.gpsimd.dma_start(out=wbc,
                            in_=stat_dram[F:F + 1, 0:F].to_broadcast([128, F]))
        lbc = singles.tile([128, F], f32)
        nc.gpsimd.dma_start(out=lbc,
                            in_=stat_dram[F:F + 1, F:2 * F].to_broadcast([128, F]))
        scbc = singles.tile([128, 2], f32)
        nc.gpsimd.dma_start(out=scbc,
                            in_=stat_dram[F + 1:F + 2, 0:2].to_broadcast([128, 2]))

        # m2 = 2M - diag(M): coefficients for the upper-triangular quadratic
        m2_sb = trans.tile([F, F], f32, tag="m2_sb", name="m2_sb")
        nc.vector.tensor_tensor(out=m2_sb, in0=m_sb, in1=ident[:F, :F],
                                op=ALU.mult)
        nc.vector.scalar_tensor_tensor(out=m2_sb, in0=m_sb, scalar=2.0,
                                       in1=m2_sb, op0=ALU.mult,
                                       op1=ALU.subtract)
        nc.sync.dma_start(out=stat_dram[1:2, 0:F * F].rearrange(
            "o (a b) -> (o a) b", a=F), in_=m2_sb)
        m2bc = singles.tile([128, F * F], f32)
        nc.gpsimd.dma_start(out=m2bc,
                            in_=stat_dram[1:2, 0:F * F].to_broadcast([128, F * F]))

        # ---------------- x load (row-major, per-q contiguous chunks) --------
        xrow_all = big.tile([128, QB, T, F], f32, tag="xrow", name="xrow_all")
        for q in range(QB):
            nc.sync.dma_start(
                out=xrow_all[:, q],
                in_=x_d[q * 128:(q + 1) * 128, :, :])

        # r_row/nmr_row computed directly in row-major [128, QB, T]
        r_row = singles.tile([128, QB, T], f32)
        nmr_row = singles.tile([128, QB, T], f32)
        nmu_r = singles.tile([128, QB, T], f32)
        QT = [128, QB, T]
        acc_f = scr.tile(QT, f32, tag="st_a", name="st_acc")
        qacc_f = scr.tile(QT, f32, tag="st_b", name="st_qacc")
        yf_f = scr.tile(QT, f32, tag="st_c", name="st_yf")
        yB_f = scr.tile(QT, f32, tag="st_e", name="st_yB")
        tmp_f = scr.tile(QT, f32, tag="st_d", name="st_tmp")
        qaccB_f = scr.tile(QT, f32, tag="st_f", name="st_qaccB")
        linB_f = scr.tile(QT, f32, tag="st_g", name="st_linB")

        def xq(fi):
            return xrow_all[:, :, :TS, fi]

        acc, qacc, yf, tmp = acc_f[:, :, :TS], qacc_f[:, :, :TS], \
            yf_f[:, :, :TS], tmp_f[:, :, :TS]
        qaccB, linB, yB = qaccB_f[:, :, :TS], linB_f[:, :, :TS], yB_f[:, :, :TS]
        # wsum.x on DVE
        nc.vector.tensor_scalar_mul(out=acc, in0=xq(0), scalar1=wbc[:, 0:1])
        for fi in range(1, F):
            nc.vector.scalar_tensor_tensor(
                out=acc, in0=xq(fi), scalar=wbc[:, fi:fi + 1],
                in1=acc, op0=ALU.mult, op1=ALU.add)
        # nmu = -(acc + bsum)/H
        nc.vector.tensor_scalar(out=nmu_r[:, :, :TS], in0=acc,
                                scalar1=scbc[:, 1:2], scalar2=-1.0 / H,
                                op0=ALU.add, op1=ALU.mult)
        # l.x on DVE
        nc.vector.tensor_scalar_mul(out=linB, in0=xq(0), scalar1=lbc[:, 0:1])
        for fi in range(1, F):
            nc.vector.scalar_tensor_tensor(
                out=linB, in0=xq(fi), scalar=lbc[:, fi:fi + 1],
                in1=linB, op0=ALU.mult, op1=ALU.add)
        # upper-tri quadratic: y_i rows on DVE (scalar ops); the x_i*y_i
        # products and the accumulation run on Pool (TensorTensor only).
        for fi in range(F):
            y_ = yf if fi % 2 == 0 else yB
            nc.vector.tensor_scalar_mul(
                out=y_, in0=xq(fi),
                scalar1=m2bc[:, fi * F + fi:fi * F + fi + 1])
            for fj in range(fi + 1, F):
                nc.vector.scalar_tensor_tensor(
                    out=y_, in0=xq(fj),
                    scalar=m2bc[:, fi * F + fj:fi * F + fj + 1],
                    in1=y_, op0=ALU.mult, op1=ALU.add)
            if fi == 0:
                nc.gpsimd.tensor_tensor(out=qaccB, in0=xq(fi), in1=y_,
                                        op=ALU.mult)
            else:
                t_ = tmp if fi % 2 == 0 else qacc
                nc.gpsimd.tensor_tensor(out=t_, in0=xq(fi), in1=y_,
                                        op=ALU.mult)
                nc.gpsimd.tensor_add(out=qaccB, in0=qaccB, in1=t_)
        # combine: qacc = qaccB + 2*linB
        nc.vector.scalar_tensor_tensor(out=qacc, in0=linB, scalar=2.0,
                                       in1=qaccB, op0=ALU.mult, op1=ALU.add)
        # var = (q + c0)/H - mu^2 ; r = 1/sqrt(var+eps)
        nc.vector.tensor_scalar(out=qacc, in0=qacc,
                                scalar1=scbc[:, 0:1], scalar2=1.0 / H,
                                op0=ALU.add, op1=ALU.mult)
        nc.vector.tensor_tensor(out=tmp, in0=nmu_r[:, :, :TS],
                                in1=nmu_r[:, :, :TS], op=ALU.mult)
        nc.vector.tensor_sub(out=qacc, in0=qacc, in1=tmp)
        nc.scalar.activation(out=r_row[:, :, :TS], in_=qacc, func=AF.Sqrt,
                             bias=eps_col, scale=1.0)
        nc.vector.reciprocal(out=r_row[:, :, :TS], in_=r_row[:, :, :TS])
        nc.vector.tensor_tensor(out=nmr_row[:, :, :TS], in0=nmu_r[:, :, :TS],
                                in1=r_row[:, :, :TS], op=ALU.mult)

        for q in range(QB):
            nc.sync.dma_start(
                out=xrow_all[:, q],
                in_=x_d[q * 128:(q + 1) * 128, :, :])

        # xrow_aug[p, q, t, 0:7] = x*r ; [...,7] = r ; [...,8] = nmr
        xrow_aug = big.tile([128, QB, T, FA], f32, tag="xtm", name="xrow_aug")
        for fi in range(F):
            nc.vector.tensor_tensor(
                out=xrow_aug[:, :, :TS, fi],
                in0=xrow_all[:, :, :TS, fi],
                in1=r_row[:, :, :TS], op=ALU.mult)
        nc.vector.tensor_copy(out=xrow_aug[:, :, :TS, F], in_=r_row[:, :, :TS])
        nc.vector.tensor_copy(out=xrow_aug[:, :, :TS, F + 1],
                              in_=nmr_row[:, :, :TS])

        # ---------------- states ----------------
        c = [[singles.tile([H, BL], f32, name=f"c{L}_{i}", tag=f"c{L}_{i}")
              for i in range(2)] for L in range(2)]
        h0_ring = [singles.tile([H, BL], bf16, name=f"h0r{i}", tag=f"h0r{i}")
                   for i in range(2)]
        h1_ring = [singles.tile([H, BL], bf16, name=f"h1r{i}", tag=f"h1r{i}")
                   for i in range(2)]
        for L in range(2):
            for i in range(2):
                nc.vector.memset(c[L][i], 0.0)
        for i in range(2):
            nc.vector.memset(h0_ring[i], 0.0)
            nc.vector.memset(h1_ring[i], 0.0)
        h1_final = singles.tile([H, BL], bf16, name="h1fin", tag="h1fin")

        # ---------------- per-step transposes + xs copies ----------------
        xs_tiles = {}

        def emit_xs_copy(t):
            """PE-transpose step t to feature-major, then DVE copies to SBUF."""
            xst = trans.tile([FA, BL], f32r, tag="xs", name=f"xs{t}")
            for half in range(2):
                pxt = ps_px.tile([FA, 512], f32, tag="px", name=f"px{t}_{half}")
                for qi in range(4):
                    q = half * 4 + qi
                    nc.tensor.transpose(pxt[:, qi * 128:(qi + 1) * 128],
                                        xrow_aug[:, q, t, :], ident)
                LBL(nc.vector.tensor_copy(
                    out=xst[:, half * 512:(half + 1) * 512], in_=pxt),
                    f"xscp{half}")
            xs_tiles[t] = xst

        emit_xs_copy(0)

        # ---------------- main loop ----------------
        # slot t: A: tanh/h for L0 step t-1 (shifted); C: L0 gates step t;
        #         D: L1 gates step t-2 with tanh/h inline at slot end.
        # ACT order/slot: tc0, tg1, si1, tg0, sf1, si0, sf0, so1, tc1, so0
        # DVE order/slot: h0, u1, v1, c1, u0, v0, c0, h1
        so_prev0 = None

        def R(ap):
            return ap.bitcast(f32r)

        n_slots = TS + 2
        for t in range(n_slots):
            do_A = 1 <= t <= TS
            do_C = t <= TS - 1
            do_D = 2 <= t <= TS + 1

            # --- A: h0_{t-1} = so0_prev * tanh(c0_{t-1}) ---
            if do_A:
                tc0 = scr.tile([H, BL], bf16, tag="tc0", name="tc0")
                LBL(nc.scalar.activation(out=tc0, in_=c[0][(t - 1) % 2],
                                         func=AF.Tanh, scale=1.0), "tc0")
                LBL(nc.vector.tensor_tensor(out=h0_ring[t % 2], in0=so_prev0,
                                            in1=tc0, op=ALU.mult), "h0")

            # --- matmuls, interleaved D/C, gate order matched to ACT order ---
            pg1 = {}
            pg0 = {}

            def emit_mm_D(gc):
                pg = pg_tile(f"pg1_{gc}")
                h0_in = h0_ring[(t - 1) % 2]
                h1_in = h1_ring[(t - 1) % 2]
                for hc in range(NH):
                    sl = slice(hc * 512, (hc + 1) * 512)
                    nc.tensor.matmul(pg[:, sl], wihT1[:, gc, :], h0_in[:, sl],
                                     start=True, stop=False,
                                     skip_group_check=True)
                    nc.tensor.matmul(pg[:, sl], whhT[1][:, gc, :], h1_in[:, sl],
                                     start=False, stop=True,
                                     skip_group_check=True)
                pg1[gc] = pg

            def emit_mm_C(gc, xst):
                pg = pg_tile(f"pg0_{gc}")
                h0_rec = h0_ring[t % 2]
                for hc in range(NH):
                    sl = slice(hc * 512, (hc + 1) * 512)
                    nc.tensor.matmul(pg[:, sl], weff_aug[:, gc, :],
                                     xst[:, sl],
                                     start=True, stop=False,
                                     skip_group_check=True)
                    nc.tensor.matmul(pg[:, sl], whhT[0][:, gc, :],
                                     h0_rec[:, sl],
                                     start=False, stop=True,
                                     skip_group_check=True)
                pg0[gc] = pg

            xst = xs_tiles.pop(t) if do_C else None
            # mm emission: D [g1,g0,g2], C [g2], D [g3], C [g0,g1,g3]
            if do_D:
                emit_mm_D(1)
                emit_mm_D(0)
                emit_mm_D(2)
            if do_C:
                emit_mm_C(2, xst)
            if do_D:
                emit_mm_D(3)
            if do_C:
                emit_mm_C(0, xst)
                emit_mm_C(1, xst)
                emit_mm_C(3, xst)

            def act_gate(pg, L, gc, dt_):
                funcs = {0: AF.Sigmoid, 1: AF.Sigmoid, 2: AF.Tanh, 3: AF.Sigmoid}
                o = scr.tile([H, BL], dt_, tag=f"g{L}_{gc}", name=f"g{L}_{gc}")
                LBL(nc.scalar.activation(out=o, in_=pg, func=funcs[gc],
                                         bias=beff[L][:, gc:gc + 1], scale=1.0),
                    f"g{L}_{gc}")
                return o

            # ACT: sf1, si1, tg1 | tg0 ; DVE: v1, u1, c1
            if do_D:
                sf1 = act_gate(pg1[1], 1, 1, f32)
                si1 = act_gate(pg1[0], 1, 0, bf16)
                tg1 = act_gate(pg1[2], 1, 2, bf16)
            if do_C:
                tg0 = act_gate(pg0[2], 0, 2, bf16)
            if do_D:
                v1 = scr.tile([H, BL], f32, tag="v1", name="v1")
                LBL(nc.vector.tensor_tensor(out=v1, in0=sf1,
                                            in1=c[1][(t - 1) % 2],
                                            op=ALU.mult), "v1")
                u1 = scr.tile([H, BL], bf16, tag="u1", name="u1")
                LBL(nc.vector.tensor_tensor(out=u1, in0=si1, in1=tg1,
                                            op=ALU.mult), "u1")
                LBL(nc.vector.tensor_add(out=c[1][t % 2], in0=u1, in1=v1), "c1")
            # ACT: si0, sf0, so1 ; DVE: u0, v0
            if do_C:
                si0 = act_gate(pg0[0], 0, 0, bf16)
                sf0 = act_gate(pg0[1], 0, 1, f32)
            if do_D:
                so1 = act_gate(pg1[3], 1, 3, bf16)
            if do_C:
                u0 = scr.tile([H, BL], bf16, tag="u0", name="u0")
                LBL(nc.vector.tensor_tensor(out=u0, in0=si0, in1=tg0,
                                            op=ALU.mult), "u0")
                v0 = scr.tile([H, BL], f32, tag="v0", name="v0")
                LBL(nc.vector.tensor_tensor(out=v0, in0=sf0,
                                            in1=c[0][(t - 1) % 2],
                                            op=ALU.mult), "v0")
            # ACT: tc1 ; DVE: c0, h1 ; ACT: so0
            if do_D:
                tc1 = scr.tile([H, BL], bf16, tag="tc1", name="tc1")
                LBL(nc.scalar.activation(out=tc1, in_=c[1][t % 2], func=AF.Tanh,
                                         scale=1.0), "tc1")
            if do_C:
                LBL(nc.vector.tensor_add(out=c[0][t % 2], in0=u0, in1=v0), "c0")
            if do_D:
                LBL(nc.vector.tensor_tensor(out=h1_ring[t % 2], in0=so1,
                                              in1=tc1, op=ALU.mult), "h1")
                if t == TS + 1:
                    nc.vector.tensor_copy(out=h1_final, in_=h1_ring[t % 2])
            if do_C:
                so0 = trans.tile([H, BL], bf16, tag="so0", name="so0")
                LBL(nc.scalar.activation(out=so0, in_=pg0[3], func=AF.Sigmoid,
                                         bias=beff[0][:, 3:4], scale=1.0), "so0")
                so_prev0 = so0

            # xs prefetch for next slot (PE transposes after the slot's mms)
            if t + 1 <= TS - 1:
                emit_xs_copy(t + 1)

        # ---------------- head ----------------
        h1f = scr.tile([H, BL], f32, tag="st_a", name="h1f")
        nc.vector.tensor_copy(out=h1f, in_=h1_final)
        sqh = scr.tile([H, BL], f32, tag="st_b", name="sqh")
        nc.vector.tensor_tensor(out=sqh, in0=h1f, in1=h1f, op=ALU.mult)
        ps_s1 = ps_pg.tile([1, BL], f32, tag="pg", name="ps_s1")
        ps_s2 = ps_pg.tile([1, BL], f32, tag="pg", name="ps_s2")
        for hc in range(NH):
            sl = slice(hc * 512, (hc + 1) * 512)
            nc.tensor.matmul(ps_s1[:, sl], ones_col, h1f[:, sl],
                             start=True, stop=True, skip_group_check=True)
            nc.tensor.matmul(ps_s2[:, sl], ones_col, sqh[:, sl],
                             start=True, stop=True, skip_group_check=True)
        nmu_h = singles.tile([1, BL], f32, tag="nmu_h", name="nmu_h")
        nc.vector.tensor_scalar_mul(out=nmu_h, in0=ps_s1, scalar1=-1.0 / H)
        musq_h = singles.tile([1, BL], f32, tag="musq", name="musq_h")
        nc.vector.tensor_tensor(out=musq_h, in0=nmu_h, in1=nmu_h, op=ALU.mult)
        v_h = singles.tile([1, BL], f32, tag="v_h", name="v_h")
        nc.vector.tensor_scalar_mul(out=v_h, in0=ps_s2, scalar1=1.0 / H)
        nc.vector.tensor_sub(out=v_h, in0=v_h, in1=musq_h)
        nc.scalar.activation(out=v_h, in_=v_h, func=AF.Sqrt,
                             bias=eps_col[0:1], scale=1.0)
        nc.vector.reciprocal(out=v_h, in_=v_h)
        hstat_dram = dpool.tile([2, BL], f32)
        nc.sync.dma_start(out=hstat_dram[0:1, :], in_=nmu_h)
        nc.sync.dma_start(out=hstat_dram[1:2, :], in_=v_h)
        nmbc = scr.tile([H, BL], f32, tag="st_c", name="nmbc")
        nc.gpsimd.dma_start(out=nmbc, in_=hstat_dram[0:1, :].to_broadcast([H, BL]))
        rhbc = scr.tile([H, BL], f32, tag="st_d", name="rhbc")
        nc.gpsimd.dma_start(out=rhbc, in_=hstat_dram[1:2, :].to_broadcast([H, BL]))
        t1 = scr.tile([H, BL], f32, tag="st_e", name="t1")
        nc.vector.tensor_tensor(out=t1, in0=h1f, in1=nmbc, op=ALU.add)
        t2 = scr.tile([H, BL], f32, tag="st_a", name="t2")
        nc.vector.tensor_tensor(out=t2, in0=t1, in1=rhbc, op=ALU.mult)
        last = scr.tile([H, BL], f32, tag="st_b", name="last")
        nc.vector.tensor_scalar(out=last, in0=t2, scalar1=g_ln_c,
                                scalar2=be_ln_c, op0=ALU.mult, op1=ALU.add)
        pd1 = pg_tile("pd1")
        for hc in range(NH):
            sl = slice(hc * 512, (hc + 1) * 512)
            nc.tensor.matmul(pd1[:D1, sl], wd1T, last[:, sl], start=True, stop=True,
                             skip_group_check=True)
        d1 = scr.tile([D1, BL], f32, tag="st_c", name="d1")
        nc.scalar.activation(out=d1, in_=pd1[:D1], func=AF.Relu, bias=b_d1_c,
                             scale=1.0)
        pd2 = pg_tile("pd2")
        for hc in range(NH):
            sl = slice(hc * 512, (hc + 1) * 512)
            nc.tensor.matmul(pd2[:D2, sl], wd2T, d1[:, sl], start=True, stop=True,
                             skip_group_check=True)
        d2 = scr.tile([D2, BL], f32, tag="st_d", name="d2")
        nc.scalar.activation(out=d2, in_=pd2[:D2], func=AF.Relu, bias=b_d2_c,
                             scale=1.0)
        pd3 = pg_tile("pd3")
        for hc in range(NH):
            sl = slice(hc * 512, (hc + 1) * 512)
            nc.tensor.matmul(pd3[:OUT, sl], wd3T, d2[:, sl], start=True, stop=True,
                             skip_group_check=True)
        o3 = scr.tile([OUT, BL], f32, tag="st_e", name="o3")
        nc.scalar.activation(out=o3, in_=pd3[:OUT], func=AF.Identity, bias=b_d3_c,
                             scale=1.0)
        outT = singles.tile([128, QB, OUT], f32)
        for q in range(QB):
            pot = px_small([128, OUT], "pot")
            nc.tensor.transpose(pot, o3[:, q * 128:(q + 1) * 128],
                                ident[:OUT, :OUT])
            nc.vector.tensor_copy(out=outT[:, q, :], in_=pot)
        nc.sync.dma_start(
            out=out_d[:, :].rearrange("(q p) c -> p q c", p=128),
            in_=outT)
        if dbg:
            h0f = scr.tile([H, BL], f32, tag="st_a", name="h0f")
            nc.vector.tensor_copy(out=h0f, in_=h0_ring[(TS) % 2])
            nc.sync.dma_start(out=dbg_h0[:, :], in_=h0f)
            nc.sync.dma_start(out=dbg_h1[:, :], in_=h1f)
    return nc


_CACHE = {}


def _get_runner():
    if "runner" in _CACHE:
        return _CACHE["runner"]
    import jax
    from jax.sharding import Mesh, PartitionSpec
    from jax.experimental.shard_map import shard_map
    import concourse.bacc as bacc
    import concourse.mybir as mybir
    from concourse.bass2jax import install_neuronx_cc_hook, _bass_exec_p, \
        partition_id_tensor

    nc = bacc.Bacc()
    _build(nc)
    nc.compile()
    install_neuronx_cc_hook()

    partition_name = nc.partition_id_tensor.name if nc.partition_id_tensor else None
    in_names, out_names, out_avals, zero_outs = [], [], [], []
    for alloc in nc.m.functions[0].allocations:
        if not isinstance(alloc, mybir.MemoryLocationSet):
            continue
        name = alloc.memorylocations[0].name
        if alloc.kind == "ExternalInput":
            if name != partition_name:
                in_names.append(name)
        elif alloc.kind == "ExternalOutput":
            out_names.append(name)
            shape = tuple(alloc.tensor_shape)
            dtype = mybir.dt.np(alloc.dtype)
            out_avals.append(jax.core.ShapedArray(shape, dtype))
            zero_outs.append(np.zeros(shape, dtype))
    n_params = len(in_names)
    all_in_names = in_names + out_names + ([partition_name] if partition_name else [])

    def _body(*args):
        operands = list(args)
        if partition_name is not None:
            operands.append(partition_id_tensor())
        outs = _bass_exec_p.bind(
            *operands,
            out_avals=tuple(out_avals),
            in_names=tuple(all_in_names),
            out_names=tuple(out_names),
            lowering_input_output_aliases=(),
            sim_require_finite=False,
            sim_require_nnan=False,
            nc=nc,
        )
        return tuple(outs)

    devices = jax.devices()[:NCORES]
    mesh = Mesh(np.asarray(devices), ("core",))
    in_specs = (PartitionSpec("core"),) * (n_params + len(out_names))
    out_specs = (PartitionSpec("core"),) * len(out_names)
    sharded = jax.jit(
        shard_map(_body, mesh=mesh, in_specs=in_specs, out_specs=out_specs,
                  check_rep=False),
        keep_unused=True)
    _CACHE["runner"] = (sharded, in_names, out_names, zero_outs)
    return _CACHE["runner"]


def kernel(**inputs) -> np.ndarray:
    sharded, in_names, out_names, zero_outs = _get_runner()
    inp = {k: np.ascontiguousarray(np.asarray(v), dtype=np.float32)
           for k, v in inputs.items()}

    def core_val(name, ci):
        if name == "x":
            return inp["x"][ci * BL:(ci + 1) * BL]
        return inp[name]

    concat_in = [
        np.concatenate([core_val(n, ci) for ci in range(NCORES)], axis=0)
        for n in in_names
    ]
    concat_zeros = [
        np.zeros((NCORES * z.shape[0], *z.shape[1:]), z.dtype) for z in zero_outs
    ]
    import jax
    out_arrs = sharded(*concat_in, *concat_zeros)
    jax.block_until_ready(out_arrs)
    oi = out_names.index("out")
    full = np.asarray(out_arrs[oi]).reshape(B, OUT)
    return full.astype(np.float32)


# revision 28
# speedup vs baseline: 1.5580x; 1.0325x over previous
"""DepletionLSTM Trainium2 kernel (v2).

Self-contained: builds a Bass/Tile kernel for the 2-layer-LSTM network,
shards the batch over 8 NeuronCores (pure data parallelism), runs via
PJRT/axon, returns the full [8192, 30] float32 output.

v2 strategy (per core, 1024 batch), ACT-bound steady state ~10.4us/step:
- W_in is folded into the layer-0 gate weights: Weff = Wih0*diag(g_in)*W_in,
  with the LN mean/rstd entering as two extra "feature" rows (r, -mu*r) of a
  9-row augmented, pre-scaled x (xs = x*r computed once in the prepass).
  This removes the per-step input projection, LN apply, x0 copy and all
  per-step DMA broadcasts.
- x transposes to feature-major are done 10 timesteps at a time (one PE
  transpose per q-chunk per window) into PSUM; per step a single [9,1024]
  Pool copy produces the matmul rhs.
- Layer 1 lags layer 0 by 2 steps; tanh(c)/h-multiply run one slot after
  their gates, so the ACT engine starts every slot with ready work and is
  the saturated bottleneck: 10 table-ops x [128,1024] per slot.
- bf16 on the h path (h, sig_i, tanh_g, sig_o, tanh_c, u, Whh/Wih1) for DVE
  2x mode and cheap recurrent matmuls; f32 for c, sig_f, v and the x path
  (float32r matmuls).
"""
import sys
sys.path.insert(0, '/opt/trn_rl_repo')

import numpy as np

B, T, F, H, D1, D2, OUT = 8192, 90, 7, 128, 128, 64, 30
NCORES = 8
BL = B // NCORES
G4 = 4 * H
NH = BL // 512
QB = BL // 128
EPS = 1e-5
W = 10            # timesteps per transpose window
NW = T // W       # 9 windows
FA = F + 2        # augmented feature rows: 7 x*r + r + (-mu*r)


LABELS = {}


def _build(nc, T_steps=T, dbg=False):
    LABELS.clear()

    def LBL(ins, label):
        try:
            LABELS[ins.ins.name if hasattr(ins, "ins") else ins.name] = label
        except Exception:
            pass
        return ins

    import concourse.tile as tile
    from concourse import mybir
    from concourse.masks import make_identity

    f32 = mybir.dt.float32
    f32r = mybir.dt.float32r
    bf16 = mybir.dt.bfloat16
    AF = mybir.ActivationFunctionType
    ALU = mybir.AluOpType

    TS = T_steps

    # ---------------- DRAM I/O ----------------
    x_d = nc.dram_tensor("x", [BL, T, F], f32, kind="ExternalInput")
    W_in_d = nc.dram_tensor("W_in", [H, F], f32, kind="ExternalInput")
    b_in_d = nc.dram_tensor("b_in", [H], f32, kind="ExternalInput")
    g_in_d = nc.dram_tensor("g_in", [H], f32, kind="ExternalInput")
    be_in_d = nc.dram_tensor("be_in", [H], f32, kind="ExternalInput")
    Wih_d = [nc.dram_tensor("Wih0", [G4, H], f32, kind="ExternalInput"),
             nc.dram_tensor("Wih1", [G4, H], f32, kind="ExternalInput")]
    Whh_d = [nc.dram_tensor("Whh0", [G4, H], f32, kind="ExternalInput"),
             nc.dram_tensor("Whh1", [G4, H], f32, kind="ExternalInput")]
    bih_d = [nc.dram_tensor("bih0", [G4], f32, kind="ExternalInput"),
             nc.dram_tensor("bih1", [G4], f32, kind="ExternalInput")]
    bhh_d = [nc.dram_tensor("bhh0", [G4], f32, kind="ExternalInput"),
             nc.dram_tensor("bhh1", [G4], f32, kind="ExternalInput")]
    g_ln_d = nc.dram_tensor("g_ln", [H], f32, kind="ExternalInput")
    be_ln_d = nc.dram_tensor("be_ln", [H], f32, kind="ExternalInput")
    W_d1_d = nc.dram_tensor("W_d1", [D1, H], f32, kind="ExternalInput")
    b_d1_d = nc.dram_tensor("b_d1", [D1], f32, kind="ExternalInput")
    W_d2_d = nc.dram_tensor("W_d2", [D2, D1], f32, kind="ExternalInput")
    b_d2_d = nc.dram_tensor("b_d2", [D2], f32, kind="ExternalInput")
    W_d3_d = nc.dram_tensor("W_d3", [OUT, D2], f32, kind="ExternalInput")
    b_d3_d = nc.dram_tensor("b_d3", [OUT], f32, kind="ExternalInput")
    out_d = nc.dram_tensor("out", [BL, OUT], f32, kind="ExternalOutput")
    if dbg:
        dbg_h0 = nc.dram_tensor("dbg_h0", [H, BL], f32, kind="ExternalOutput")
        dbg_h1 = nc.dram_tensor("dbg_h1", [H, BL], f32, kind="ExternalOutput")
        dbg_xs = nc.dram_tensor("dbg_xs", [FA, BL], f32, kind="ExternalOutput")

    import contextlib
    with tile.TileContext(nc) as tc, contextlib.ExitStack() as ctx:
        singles = ctx.enter_context(tc.tile_pool(name="singles", bufs=1))
        big = ctx.enter_context(tc.tile_pool(name="big", bufs=1))
        trans = ctx.enter_context(tc.tile_pool(name="trans", bufs=2))
        scr = ctx.enter_context(tc.tile_pool(name="scr", bufs=1))
        ps_pg = ctx.enter_context(tc.tile_pool(name="ps_pg", bufs=3, space="PSUM"))
        ps_px = ctx.enter_context(tc.tile_pool(name="ps_px", bufs=2, space="PSUM"))
        dpool = ctx.enter_context(tc.tile_pool(name="dpool", bufs=1, space="DRAM"))

        def pg_tile(name):
            return ps_pg.tile([H, BL], f32, tag="pg", name=name)

        def px_small(shape, name):
            return ps_px.tile(shape, f32, tag="px", name=name)

        # ---------------- constants ----------------
        ident = singles.tile([128, 128], f32)
        make_identity(nc, ident)
        ones_row = singles.tile([1, 512], f32)
        nc.vector.memset(ones_row, 1.0)
        ones_col = singles.tile([128, 1], f32)
        nc.vector.memset(ones_col, 1.0)
        eps_col = singles.tile([128, 1], f32)
        nc.vector.memset(eps_col, EPS)

        def load_col(dram_vec, n, name):
            t_ = singles.tile([n, 1], f32, name=name, tag=name)
            nc.sync.dma_start(out=t_, in_=dram_vec[:].rearrange("(p o) -> p o", o=1))
            return t_

        g_in_c = load_col(g_in_d, H, "g_in_c")
        be_in_c = load_col(be_in_d, H, "be_in_c")
        b_in_c = load_col(b_in_d, H, "b_in_c")
        g_ln_c = load_col(g_ln_d, H, "g_ln_c")
        be_ln_c = load_col(be_ln_d, H, "be_ln_c")
        b_d1_c = load_col(b_d1_d, D1, "b_d1_c")
        b_d2_c = load_col(b_d2_d, D2, "b_d2_c")
        b_d3_c = load_col(b_d3_d, OUT, "b_d3_c")

        w_in_raw = singles.tile([H, F], f32)
        nc.sync.dma_start(out=w_in_raw, in_=W_in_d[:, :])

        # ---------------- prepass: LN stats in [T, BL] layout ----------------
        # p' = W_in x + b_in per (h | b,t); over h:
        #   sum p'   = wsum . x + bsum
        #   sum p'^2 = x^T M x + 2 l^T x + c0,  M = W^T W, l = W^T b, c0=|b|^2
        p_m = px_small([F, F], "stat_m")
        nc.tensor.matmul(p_m, w_in_raw, w_in_raw, start=True, stop=True)
        p_ws = px_small([1, F], "stat_ws")
        nc.tensor.matmul(p_ws, ones_col, w_in_raw, start=True, stop=True)
        p_l = px_small([1, F], "stat_l")
        nc.tensor.matmul(p_l, b_in_c, w_in_raw, start=True, stop=True)
        p_sc = px_small([1, 2], "stat_sc")
        nc.tensor.matmul(p_sc[:, 0:1], b_in_c, b_in_c, start=True, stop=False,
                         skip_group_check=True)
        nc.tensor.matmul(p_sc[:, 1:2], ones_col, b_in_c, start=False, stop=True,
                         skip_group_check=True)
        m_sb = trans.tile([F, F], f32, tag="m_sb", name="m_sb")
        nc.vector.tensor_copy(out=m_sb, in_=p_m)
        ws_sb = trans.tile([1, F], f32, tag="ws_sb", name="ws_sb")
        nc.vector.tensor_copy(out=ws_sb, in_=p_ws)
        l_sb = trans.tile([1, F], f32, tag="l_sb", name="l_sb")
        nc.vector.tensor_copy(out=l_sb, in_=p_l)
        sc_sb = trans.tile([1, 2], f32, tag="sc_sb", name="sc_sb")
        nc.vector.tensor_copy(out=sc_sb, in_=p_sc)
        # stage stat constants to DRAM, then partition-broadcast them back
        stat_dram = dpool.tile([F + 2, F * F], f32)
        nc.sync.dma_start(out=stat_dram[0:1, :].rearrange("o (a b) -> (o a) b", a=F),
                          in_=m_sb)
        nc.sync.dma_start(out=stat_dram[F:F + 1, 0:F], in_=ws_sb)
        nc.sync.dma_start(out=stat_dram[F:F + 1, F:2 * F], in_=l_sb)
        nc.sync.dma_start(out=stat_dram[F + 1:F + 2, 0:2], in_=sc_sb)
        wbc = singles.tile([128, F], f32)
        nc.gpsimd.dma_start(out=wbc,
                            in_=stat_dram[F:F + 1, 0:F].to_broadcast([128, F]))
        lbc = singles.tile([128, F], f32)
        nc.gpsimd.dma_start(out=lbc,
                            in_=stat_dram[F:F + 1, F:2 * F].to_broadcast([128, F]))
        scbc = singles.tile([128, 2], f32)
        nc.gpsimd.dma_start(out=scbc,
                            in_=stat_dram[F + 1:F + 2, 0:2].to_broadcast([128, 2]))

        # m2 = 2M - diag(M): coefficients for the upper-triangular quadratic
        m2_sb = trans.tile([F, F], f32, tag="m2_sb", name="m2_sb")
        nc.vector.tensor_tensor(out=m2_sb, in0=m_sb, in1=ident[:F, :F],
                                op=ALU.mult)
        nc.vector.scalar_tensor_tensor(out=m2_sb, in0=m_sb, scalar=2.0,
                                       in1=m2_sb, op0=ALU.mult,
                                       op1=ALU.subtract)
        nc.sync.dma_start(out=stat_dram[1:2, 0:F * F].rearrange(
            "o (a b) -> (o a) b", a=F), in_=m2_sb)
        m2bc = singles.tile([128, F * F], f32)
        nc.gpsimd.dma_start(out=m2bc,
                            in_=stat_dram[1:2, 0:F * F].to_broadcast([128, F * F]))

        # ---------------- x load (row-major, per-q contiguous chunks) --------
        xrow_all = big.tile([128, QB, T, F], f32, tag="xrow", name="xrow_all")
        for q in range(QB):
            nc.sync.dma_start(
                out=xrow_all[:, q],
                in_=x_d[q * 128:(q + 1) * 128, :, :])

        # r_row/nmr_row computed directly in row-major [128, QB, T]
        r_row = singles.tile([128, QB, T], f32)
        nmr_row = singles.tile([128, QB, T], f32)
        nmu_r = singles.tile([128, QB, T], f32)
        QT = [128, QB, T]
        acc_f = scr.tile(QT, f32, tag="st_a", name="st_acc")
        qacc_f = scr.tile(QT, f32, tag="st_b", name="st_qacc")
        yf_f = scr.tile(QT, f32, tag="st_c", name="st_yf")
        yB_f = scr.tile(QT, f32, tag="st_e", name="st_yB")
        tmp_f = scr.tile(QT, f32, tag="st_d", name="st_tmp")
        qaccB_f = scr.tile(QT, f32, tag="st_f", name="st_qaccB")
        linB_f = scr.tile(QT, f32, tag="st_g", name="st_linB")

        def xq(fi):
            return xrow_all[:, :, :TS, fi]

        acc, qacc, yf, tmp = acc_f[:, :, :TS], qacc_f[:, :, :TS], \
            yf_f[:, :, :TS], tmp_f[:, :, :TS]
        qaccB, linB, yB = qaccB_f[:, :, :TS], linB_f[:, :, :TS], yB_f[:, :, :TS]
        # wsum.x on DVE
        nc.vector.tensor_scalar_mul(out=acc, in0=xq(0), scalar1=wbc[:, 0:1])
        for fi in range(1, F):
            nc.vector.scalar_tensor_tensor(
                out=acc, in0=xq(fi), scalar=wbc[:, fi:fi + 1],
                in1=acc, op0=ALU.mult, op1=ALU.add)
        # nmu = -(acc + bsum)/H
        nc.vector.tensor_scalar(out=nmu_r[:, :, :TS], in0=acc,
                                scalar1=scbc[:, 1:2], scalar2=-1.0 / H,
                                op0=ALU.add, op1=ALU.mult)
        # l.x on DVE
        nc.vector.tensor_scalar_mul(out=linB, in0=xq(0), scalar1=lbc[:, 0:1])
        for fi in range(1, F):
            nc.vector.scalar_tensor_tensor(
                out=linB, in0=xq(fi), scalar=lbc[:, fi:fi + 1],
                in1=linB, op0=ALU.mult, op1=ALU.add)
        # upper-tri quadratic: y_i rows on DVE (scalar ops); the x_i*y_i
        # products and the accumulation run on Pool (TensorTensor only).
        ybufs = [yf, yB, tmp, qacc]
        for fi in range(F):
            y_ = ybufs[fi % 4]
            nc.vector.tensor_scalar_mul(
                out=y_, in0=xq(fi),
                scalar1=m2bc[:, fi * F + fi:fi * F + fi + 1])
            for fj in range(fi + 1, F):
                nc.vector.scalar_tensor_tensor(
                    out=y_, in0=xq(fj),
                    scalar=m2bc[:, fi * F + fj:fi * F + fj + 1],
                    in1=y_, op0=ALU.mult, op1=ALU.add)
            if fi == 0:
                nc.gpsimd.tensor_tensor(out=qaccB, in0=xq(fi), in1=y_,
                                        op=ALU.mult)
            else:
                nc.gpsimd.tensor_tensor(out=y_, in0=xq(fi), in1=y_,
                                        op=ALU.mult)
                nc.gpsimd.tensor_add(out=qaccB, in0=qaccB, in1=y_)
        # combine: qacc = qaccB + 2*linB
        nc.vector.scalar_tensor_tensor(out=qacc, in0=linB, scalar=2.0,
                                       in1=qaccB, op0=ALU.mult, op1=ALU.add)
        # var = (q + c0)/H - mu^2 ; r = 1/sqrt(var+eps)
        nc.vector.tensor_scalar(out=qacc, in0=qacc,
                                scalar1=scbc[:, 0:1], scalar2=1.0 / H,
                                op0=ALU.add, op1=ALU.mult)
        nc.vector.tensor_tensor(out=tmp, in0=nmu_r[:, :, :TS],
                                in1=nmu_r[:, :, :TS], op=ALU.mult)
        nc.vector.tensor_sub(out=qacc, in0=qacc, in1=tmp)
        nc.scalar.activation(out=r_row[:, :, :TS], in_=qacc, func=AF.Sqrt,
                             bias=eps_col, scale=1.0)
        nc.vector.reciprocal(out=r_row[:, :, :TS], in_=r_row[:, :, :TS])
        nc.vector.tensor_tensor(out=nmr_row[:, :, :TS], in0=nmu_r[:, :, :TS],
                                in1=r_row[:, :, :TS], op=ALU.mult)

        for q in range(QB):
            nc.sync.dma_start(
                out=xrow_all[:, q],
                in_=x_d[q * 128:(q + 1) * 128, :, :])

        # ---------------- weights: load + PE-transpose ----------------
        def transpose_to(dst, src_ap, p, fdim):
            pt = px_small([fdim, p], "tr_ps")
            nc.tensor.transpose(pt, src_ap, ident[:p, :p])
            nc.vector.tensor_copy(out=dst, in_=pt)

        # wihT0g: transposed Wih0 with gamma fold (f32, rhs for Weff build)
        # wihT0f: transposed Wih0 pre-gamma (for bias beta-fold)
        wihT0g = singles.tile([H, 4, H], f32)
        wihT0f = singles.tile([H, 4, H], f32)
        # bf16 recurrent weights
        wihT1 = singles.tile([H, 4, H], bf16, name="wihT1", tag="wihT1")
        whhT = [singles.tile([H, 4, H], bf16, name=f"whhT{L}", tag=f"whhT{L}")
                for L in range(2)]
        for cc in range(4):
            raw = trans.tile([H, H], f32, tag="wraw", name="raw")
            nc.sync.dma_start(out=raw, in_=Wih_d[0][cc * H:(cc + 1) * H, :])
            pt_w = px_small([H, H], "tr_ps")
            nc.tensor.transpose(pt_w, raw, ident)
            nc.vector.tensor_copy(out=wihT0f[:, cc, :], in_=pt_w)
            raw2 = trans.tile([H, H], f32, tag="wraw2", name="raw2")
            nc.sync.dma_start(out=raw2, in_=Wih_d[1][cc * H:(cc + 1) * H, :])
            transpose_to(wihT1[:, cc, :], raw2, H, H)
            for L in range(2):
                raw3 = trans.tile([H, H], f32, tag="wraw", name="raw3")
                nc.sync.dma_start(out=raw3, in_=Whh_d[L][cc * H:(cc + 1) * H, :])
                transpose_to(whhT[L][:, cc, :], raw3, H, H)
        # gamma fold: wihT0g[p, c, m] = Wih0[c*128+m, p] * g_in[p]
        nc.vector.tensor_scalar_mul(
            out=wihT0g[:, :, :].rearrange("p c m -> p (c m)"),
            in0=wihT0f[:, :, :].rearrange("p c m -> p (c m)"),
            scalar1=g_in_c)

        # gate biases beff[L] [128, 4]; layer-0 gets +Wih0 @ be_in (beta fold)
        beff = []
        for L in range(2):
            bt_ = singles.tile([H, 4], f32, name=f"beff{L}", tag=f"beff{L}")
            bih_sb = trans.tile([H, 4], f32, tag="bload", name="bih_sb")
            nc.sync.dma_start(out=bih_sb,
                              in_=bih_d[L][:].rearrange("(c p) -> p c", p=H))
            bhh_sb = trans.tile([H, 4], f32, tag="bload2", name="bhh_sb")
            nc.sync.dma_start(out=bhh_sb,
                              in_=bhh_d[L][:].rearrange("(c p) -> p c", p=H))
            nc.vector.tensor_add(out=bt_, in0=bih_sb, in1=bhh_sb)
            beff.append(bt_)
        for cc in range(4):
            pb = px_small([H, 1], "pb")
            nc.tensor.matmul(pb, wihT0f[:, cc, :], be_in_c, start=True, stop=True)
            nc.vector.tensor_add(out=beff[0][:, cc:cc + 1],
                                 in0=beff[0][:, cc:cc + 1], in1=pb)

        # weff_aug [FA=9, 4, 128] f32 (used via bitcast f32r):
        #  rows 0-6 = (Wih0*diag(g)*W_in)^T ; row 7 = Wih0@(g*b_in); row 8 = Wih0@g
        weff_aug = singles.tile([FA, 4, H], f32r)
        gb_in_c = singles.tile([H, 1], f32)
        nc.vector.tensor_tensor(out=gb_in_c, in0=b_in_c, in1=g_in_c, op=ALU.mult)
        weff_dram = dpool.tile([FA, 4 * H], f32)
        wtmp7 = trans.tile([F, 4 * H], f32, tag="wtmp7", name="wtmp7")
        wtmp1 = trans.tile([1, 4 * H], f32, tag="wtmp1", name="wtmp1")
        wtmp2 = trans.tile([1, 4 * H], f32, tag="wtmp2", name="wtmp2")
        for cc in range(4):
            pwe = px_small([F, H], "pwe")
            nc.tensor.matmul(pwe, w_in_raw, wihT0g[:, cc, :], start=True, stop=True)
            nc.vector.tensor_copy(out=wtmp7[:, cc * H:(cc + 1) * H], in_=pwe)
            pb1 = px_small([1, H], "pb1")
            nc.tensor.matmul(pb1, gb_in_c, wihT0g[:, cc, :], start=True, stop=True)
            nc.vector.tensor_copy(out=wtmp1[:, cc * H:(cc + 1) * H], in_=pb1)
            pb2 = px_small([1, H], "pb2")
            nc.tensor.matmul(pb2, g_in_c, wihT0g[:, cc, :], start=True, stop=True)
            nc.vector.tensor_copy(out=wtmp2[:, cc * H:(cc + 1) * H], in_=pb2)
        nc.sync.dma_start(out=weff_dram[0:F, :], in_=wtmp7)
        nc.sync.dma_start(out=weff_dram[F:F + 1, :], in_=wtmp1)
        nc.sync.dma_start(out=weff_dram[F + 1:F + 2, :], in_=wtmp2)
        weff_stage = trans.tile([FA, 4 * H], f32, tag="weff_stage",
                                name="weff_stage")
        nc.sync.dma_start(out=weff_stage, in_=weff_dram[:, :])
        nc.vector.tensor_copy(
            out=weff_aug[:, :, :].rearrange("p c m -> p (c m)"),
            in_=weff_stage)

        wd1T = singles.tile([H, D1], f32r)
        wd1_raw = trans.tile([D1, H], f32, tag="wraw", name="wd1_raw")
        nc.sync.dma_start(out=wd1_raw, in_=W_d1_d[:, :])
        transpose_to(wd1T, wd1_raw, D1, H)
        wd2T = singles.tile([D1, D2], f32r)
        wd2_raw = trans.tile([D2, D1], f32, tag="wraw2", name="wd2_raw")
        nc.sync.dma_start(out=wd2_raw, in_=W_d2_d[:, :])
        transpose_to(wd2T, wd2_raw, D2, D1)
        wd3T = singles.tile([D2, OUT], f32r)
        wd3_raw = trans.tile([OUT, D2], f32, tag="wraw", name="wd3_raw")
        nc.sync.dma_start(out=wd3_raw, in_=W_d3_d[:, :])
        transpose_to(wd3T, wd3_raw, OUT, D2)

        # xrow_aug[p, q, t, 0:7] = x*r ; [...,7] = r ; [...,8] = nmr
        xrow_aug = big.tile([128, QB, T, FA], f32, tag="xtm", name="xrow_aug")
        for fi in range(F):
            nc.vector.tensor_tensor(
                out=xrow_aug[:, :, :TS, fi],
                in0=xrow_all[:, :, :TS, fi],
                in1=r_row[:, :, :TS], op=ALU.mult)
        nc.vector.tensor_copy(out=xrow_aug[:, :, :TS, F], in_=r_row[:, :, :TS])
        nc.vector.tensor_copy(out=xrow_aug[:, :, :TS, F + 1],
                              in_=nmr_row[:, :, :TS])

        # ---------------- states ----------------
        c = [[singles.tile([H, BL], f32, name=f"c{L}_{i}", tag=f"c{L}_{i}")
              for i in range(2)] for L in range(2)]
        h0_ring = [singles.tile([H, BL], bf16, name=f"h0r{i}", tag=f"h0r{i}")
                   for i in range(2)]
        h1_ring = [singles.tile([H, BL], bf16, name=f"h1r{i}", tag=f"h1r{i}")
                   for i in range(2)]
        for L in range(2):
            for i in range(2):
                nc.vector.memset(c[L][i], 0.0)
        for i in range(2):
            nc.vector.memset(h0_ring[i], 0.0)
            nc.vector.memset(h1_ring[i], 0.0)
        h1_final = singles.tile([H, BL], bf16, name="h1fin", tag="h1fin")

        # ---------------- per-step transposes + xs copies ----------------
        xs_tiles = {}

        def emit_xs_copy(t):
            """PE-transpose step t to feature-major, then DVE copies to SBUF."""
            xst = trans.tile([FA, BL], f32r, tag="xs", name=f"xs{t}")
            for half in range(2):
                pxt = ps_px.tile([FA, 512], f32, tag="px", name=f"px{t}_{half}")
                for qi in range(4):
                    q = half * 4 + qi
                    nc.tensor.transpose(pxt[:, qi * 128:(qi + 1) * 128],
                                        xrow_aug[:, q, t, :], ident)
                LBL(nc.vector.tensor_copy(
                    out=xst[:, half * 512:(half + 1) * 512], in_=pxt),
                    f"xscp{half}")
            xs_tiles[t] = xst

        emit_xs_copy(0)

        # ---------------- main loop ----------------
        # slot t: A: tanh/h for L0 step t-1 (shifted); C: L0 gates step t;
        #         D: L1 gates step t-2 with tanh/h inline at slot end.
        # ACT order/slot: tc0, tg1, si1, tg0, sf1, si0, sf0, so1, tc1, so0
        # DVE order/slot: h0, u1, v1, c1, u0, v0, c0, h1
        so_prev0 = None

        def R(ap):
            return ap.bitcast(f32r)

        n_slots = TS + 2
        for t in range(n_slots):
            do_A = 1 <= t <= TS
            do_C = t <= TS - 1
            do_D = 2 <= t <= TS + 1

            # --- A: h0_{t-1} = so0_prev * tanh(c0_{t-1}) ---
            if do_A:
                tc0 = scr.tile([H, BL], bf16, tag="tc0", name="tc0")
                LBL(nc.scalar.activation(out=tc0, in_=c[0][(t - 1) % 2],
                                         func=AF.Tanh, scale=1.0), "tc0")
                LBL(nc.vector.tensor_tensor(out=h0_ring[t % 2], in0=so_prev0,
                                            in1=tc0, op=ALU.mult), "h0")

            # --- matmuls, interleaved D/C, gate order matched to ACT order ---
            pg1 = {}
            pg0 = {}

            def emit_mm_D(gc):
                pg = pg_tile(f"pg1_{gc}")
                h0_in = h0_ring[(t - 1) % 2]
                h1_in = h1_ring[(t - 1) % 2]
                for hc in range(NH):
                    sl = slice(hc * 512, (hc + 1) * 512)
                    nc.tensor.matmul(pg[:, sl], wihT1[:, gc, :], h0_in[:, sl],
                                     start=True, stop=False,
                                     skip_group_check=True)
                    nc.tensor.matmul(pg[:, sl], whhT[1][:, gc, :], h1_in[:, sl],
                                     start=False, stop=True,
                                     skip_group_check=True)
                pg1[gc] = pg

            def emit_mm_C(gc, xst):
                pg = pg_tile(f"pg0_{gc}")
                h0_rec = h0_ring[t % 2]
                for hc in range(NH):
                    sl = slice(hc * 512, (hc + 1) * 512)
                    nc.tensor.matmul(pg[:, sl], weff_aug[:, gc, :],
                                     xst[:, sl],
                                     start=True, stop=False,
                                     skip_group_check=True)
                    nc.tensor.matmul(pg[:, sl], whhT[0][:, gc, :],
                                     h0_rec[:, sl],
                                     start=False, stop=True,
                                     skip_group_check=True)
                pg0[gc] = pg

            xst = xs_tiles.pop(t) if do_C else None
            # mm emission: D [g1,g0,g2], C [g2], D [g3], C [g0,g1,g3]
            if do_D:
                emit_mm_D(1)
                emit_mm_D(0)
                emit_mm_D(2)
            if do_C:
                emit_mm_C(2, xst)
            if do_D:
                emit_mm_D(3)
            if do_C:
                emit_mm_C(0, xst)
                emit_mm_C(1, xst)
                emit_mm_C(3, xst)

            def act_gate(pg, L, gc, dt_):
                funcs = {0: AF.Sigmoid, 1: AF.Sigmoid, 2: AF.Tanh, 3: AF.Sigmoid}
                o = scr.tile([H, BL], dt_, tag=f"g{L}_{gc}", name=f"g{L}_{gc}")
                LBL(nc.scalar.activation(out=o, in_=pg, func=funcs[gc],
                                         bias=beff[L][:, gc:gc + 1], scale=1.0),
                    f"g{L}_{gc}")
                return o

            # ACT: sf1, si1, tg1 | tg0 ; DVE: v1, u1, c1
            if do_D:
                sf1 = act_gate(pg1[1], 1, 1, f32)
                si1 = act_gate(pg1[0], 1, 0, bf16)
                tg1 = act_gate(pg1[2], 1, 2, bf16)
            if do_C:
                tg0 = act_gate(pg0[2], 0, 2, bf16)
            if do_D:
                v1 = scr.tile([H, BL], f32, tag="v1", name="v1")
                LBL(nc.vector.tensor_tensor(out=v1, in0=sf1,
                                            in1=c[1][(t - 1) % 2],
                                            op=ALU.mult), "v1")
                u1 = scr.tile([H, BL], bf16, tag="u1", name="u1")
                LBL(nc.vector.tensor_tensor(out=u1, in0=si1, in1=tg1,
                                            op=ALU.mult), "u1")
                LBL(nc.vector.tensor_add(out=c[1][t % 2], in0=u1, in1=v1), "c1")
            # ACT: si0, sf0, so1 ; DVE: u0, v0
            if do_C:
                si0 = act_gate(pg0[0], 0, 0, bf16)
                sf0 = act_gate(pg0[1], 0, 1, f32)
            if do_D:
                so1 = act_gate(pg1[3], 1, 3, bf16)
            if do_C:
                u0 = scr.tile([H, BL], bf16, tag="u0", name="u0")
                LBL(nc.vector.tensor_tensor(out=u0, in0=si0, in1=tg0,
                                            op=ALU.mult), "u0")
                v0 = scr.tile([H, BL], f32, tag="v0", name="v0")
                LBL(nc.vector.tensor_tensor(out=v0, in0=sf0,
                                            in1=c[0][(t - 1) % 2],
                                            op=ALU.mult), "v0")
            # ACT: tc1 ; DVE: c0, h1 ; ACT: so0
            if do_D:
                tc1 = scr.tile([H, BL], bf16, tag="tc1", name="tc1")
                LBL(nc.scalar.activation(out=tc1, in_=c[1][t % 2], func=AF.Tanh,
                                         scale=1.0), "tc1")
            if do_C:
                LBL(nc.vector.tensor_add(out=c[0][t % 2], in0=u0, in1=v0), "c0")
            if do_D:
                LBL(nc.vector.tensor_tensor(out=h1_ring[t % 2], in0=so1,
                                              in1=tc1, op=ALU.mult), "h1")
                if t == TS + 1:
                    nc.vector.tensor_copy(out=h1_final, in_=h1_ring[t % 2])
            if do_C:
                so0 = trans.tile([H, BL], bf16, tag="so0", name="so0")
                LBL(nc.scalar.activation(out=so0, in_=pg0[3], func=AF.Sigmoid,
                                         bias=beff[0][:, 3:4], scale=1.0), "so0")
                so_prev0 = so0

            # xs prefetch for next slot (PE transposes after the slot's mms)
            if t + 1 <= TS - 1:
                emit_xs_copy(t + 1)

        # ---------------- head ----------------
        h1f = scr.tile([H, BL], f32r, tag="st_a", name="h1f")
        nc.vector.tensor_copy(out=h1f, in_=h1_final)
        sqh = scr.tile([H, BL], f32r, tag="st_b", name="sqh")
        nc.vector.tensor_tensor(out=sqh, in0=h1f, in1=h1f, op=ALU.mult)
        ones_col_r = singles.tile([128, 1], f32r)
        nc.vector.tensor_copy(out=ones_col_r, in_=ones_col)
        ones_row_r = singles.tile([1, 128], f32r)
        nc.vector.tensor_copy(out=ones_row_r, in_=ones_row[0:1, 0:128])
        ps_s1 = ps_pg.tile([1, BL], f32, tag="pg", name="ps_s1")
        ps_s2 = ps_pg.tile([1, BL], f32, tag="pg", name="ps_s2")
        for hc in range(NH):
            sl = slice(hc * 512, (hc + 1) * 512)
            nc.tensor.matmul(ps_s1[:, sl], ones_col_r, h1f[:, sl],
                             start=True, stop=True, skip_group_check=True)
            nc.tensor.matmul(ps_s2[:, sl], ones_col_r, sqh[:, sl],
                             start=True, stop=True, skip_group_check=True)
        nmu_h = singles.tile([1, BL], f32r, tag="nmu_h", name="nmu_h")
        nc.vector.tensor_scalar_mul(out=nmu_h, in0=ps_s1, scalar1=-1.0 / H)
        musq_h = singles.tile([1, BL], f32, tag="musq", name="musq_h")
        nc.vector.tensor_tensor(out=musq_h, in0=nmu_h, in1=nmu_h, op=ALU.mult)
        v_h = singles.tile([1, BL], f32, tag="v_h", name="v_h")
        nc.vector.tensor_scalar_mul(out=v_h, in0=ps_s2, scalar1=1.0 / H)
        nc.vector.tensor_sub(out=v_h, in0=v_h, in1=musq_h)
        nc.scalar.activation(out=v_h, in_=v_h, func=AF.Sqrt,
                             bias=eps_col[0:1], scale=1.0)
        v_hr = singles.tile([1, BL], f32r, tag="v_hr", name="v_hr")
        with nc.allow_low_precision(reason="f32r rounding for PE broadcast"):
            nc.vector.reciprocal(out=v_hr, in_=v_h)
        # broadcast LN stats across partitions via PE outer products
        pnm = ps_pg.tile([H, BL], f32, tag="pg", name="pnm")
        prh = ps_pg.tile([H, BL], f32, tag="pg", name="prh")
        for hc in range(NH):
            sl = slice(hc * 512, (hc + 1) * 512)
            nc.tensor.matmul(pnm[:, sl], ones_row_r, nmu_h[:, sl],
                             start=True, stop=True, skip_group_check=True)
            nc.tensor.matmul(prh[:, sl], ones_row_r, v_hr[:, sl],
                             start=True, stop=True, skip_group_check=True)
        t1 = scr.tile([H, BL], f32, tag="st_c", name="t1")
        nc.vector.tensor_tensor(out=t1, in0=h1f, in1=pnm, op=ALU.add)
        t2 = scr.tile([H, BL], f32, tag="st_d", name="t2")
        nc.vector.tensor_tensor(out=t2, in0=t1, in1=prh, op=ALU.mult)
        last = scr.tile([H, BL], f32r, tag="st_e", name="last")
        nc.vector.tensor_scalar(out=last, in0=t2, scalar1=g_ln_c,
                                scalar2=be_ln_c, op0=ALU.mult, op1=ALU.add)
        pd1 = pg_tile("pd1")
        for hc in range(NH):
            sl = slice(hc * 512, (hc + 1) * 512)
            nc.tensor.matmul(pd1[:D1, sl], wd1T, last[:, sl], start=True,
                             stop=True, skip_group_check=True)
        d1 = scr.tile([D1, BL], f32r, tag="st_a", name="d1")
        nc.scalar.activation(out=d1, in_=pd1[:D1], func=AF.Relu, bias=b_d1_c,
                             scale=1.0)
        pd2 = pg_tile("pd2")
        for hc in range(NH):
            sl = slice(hc * 512, (hc + 1) * 512)
            nc.tensor.matmul(pd2[:D2, sl], wd2T, d1[:, sl], start=True,
                             stop=True, skip_group_check=True)
        d2 = scr.tile([D2, BL], f32r, tag="st_b", name="d2")
        nc.scalar.activation(out=d2, in_=pd2[:D2], func=AF.Relu, bias=b_d2_c,
                             scale=1.0)
        pd3 = pg_tile("pd3")
        for hc in range(NH):
            sl = slice(hc * 512, (hc + 1) * 512)
            nc.tensor.matmul(pd3[:OUT, sl], wd3T, d2[:, sl], start=True,
                             stop=True, skip_group_check=True)
        o3 = scr.tile([OUT, BL], f32, tag="st_c", name="o3")
        nc.scalar.activation(out=o3, in_=pd3[:OUT], func=AF.Identity,
                             bias=b_d3_c, scale=1.0)
        outT = singles.tile([128, QB, OUT], f32)
        for q in range(QB):
            pot = px_small([128, OUT], "pot")
            nc.tensor.transpose(pot, o3[:, q * 128:(q + 1) * 128],
                                ident[:OUT, :OUT])
            nc.vector.tensor_copy(out=outT[:, q, :], in_=pot)
        nc.sync.dma_start(
            out=out_d[:, :].rearrange("(q p) c -> p q c", p=128),
            in_=outT)
        if dbg:
            h0f = scr.tile([H, BL], f32, tag="st_a", name="h0f")
            nc.vector.tensor_copy(out=h0f, in_=h0_ring[(TS) % 2])
            nc.sync.dma_start(out=dbg_h0[:, :], in_=h0f)
            nc.sync.dma_start(out=dbg_h1[:, :], in_=h1f)
    return nc


_CACHE = {}


def _get_runner():
    if "runner" in _CACHE:
        return _CACHE["runner"]
    import jax
    from jax.sharding import Mesh, PartitionSpec
    from jax.experimental.shard_map import shard_map
    import concourse.bacc as bacc
    import concourse.mybir as mybir
    from concourse.bass2jax import install_neuronx_cc_hook, _bass_exec_p, \
        partition_id_tensor

    nc = bacc.Bacc()
    _build(nc)
    nc.compile()
    install_neuronx_cc_hook()

    partition_name = nc.partition_id_tensor.name if nc.partition_id_tensor else None
    in_names, out_names, out_avals, zero_outs = [], [], [], []
    for alloc in nc.m.functions[0].allocations:
        if not isinstance(alloc, mybir.MemoryLocationSet):
            continue
        name = alloc.memorylocations[0].name
        if alloc.kind == "ExternalInput":
            if name != partition_name:
                in_names.append(name)
        elif alloc.kind == "ExternalOutput":
            out_names.append(name)
            shape = tuple(alloc.tensor_shape)
            dtype = mybir.dt.np(alloc.dtype)
            out_avals.append(jax.core.ShapedArray(shape, dtype))
            zero_outs.append(np.zeros(shape, dtype))
    n_params = len(in_names)
    all_in_names = in_names + out_names + ([partition_name] if partition_name else [])

    def _body(*args):
        operands = list(args)
        if partition_name is not None:
            operands.append(partition_id_tensor())
        outs = _bass_exec_p.bind(
            *operands,
            out_avals=tuple(out_avals),
            in_names=tuple(all_in_names),
            out_names=tuple(out_names),
            lowering_input_output_aliases=(),
            sim_require_finite=False,
            sim_require_nnan=False,
            nc=nc,
        )
        return tuple(outs)

    devices = jax.devices()[:NCORES]
    mesh = Mesh(np.asarray(devices), ("core",))
    in_specs = (PartitionSpec("core"),) * (n_params + len(out_names))
    out_specs = (PartitionSpec("core"),) * len(out_names)
    sharded = jax.jit(
        shard_map(_body, mesh=mesh, in_specs=in_specs, out_specs=out_specs,
                  check_rep=False),
        keep_unused=True)
    _CACHE["runner"] = (sharded, in_names, out_names, zero_outs)
    return _CACHE["runner"]


def kernel(**inputs) -> np.ndarray:
    sharded, in_names, out_names, zero_outs = _get_runner()
    inp = {k: np.ascontiguousarray(np.asarray(v), dtype=np.float32)
           for k, v in inputs.items()}

    def core_val(name, ci):
        if name == "x":
            return inp["x"][ci * BL:(ci + 1) * BL]
        return inp[name]

    concat_in = [
        np.concatenate([core_val(n, ci) for ci in range(NCORES)], axis=0)
        for n in in_names
    ]
    concat_zeros = [
        np.zeros((NCORES * z.shape[0], *z.shape[1:]), z.dtype) for z in zero_outs
    ]
    import jax
    out_arrs = sharded(*concat_in, *concat_zeros)
    jax.block_until_ready(out_arrs)
    oi = out_names.index("out")
    full = np.asarray(out_arrs[oi]).reshape(B, OUT)
    return full.astype(np.float32)


# revision 34
# speedup vs baseline: 1.5673x; 1.0060x over previous
"""DepletionLSTM Trainium2 kernel (v2).

Self-contained: builds a Bass/Tile kernel for the 2-layer-LSTM network,
shards the batch over 8 NeuronCores (pure data parallelism), runs via
PJRT/axon, returns the full [8192, 30] float32 output.

v2 strategy (per core, 1024 batch), ACT-bound steady state ~10.4us/step:
- W_in is folded into the layer-0 gate weights: Weff = Wih0*diag(g_in)*W_in,
  with the LN mean/rstd entering as two extra "feature" rows (r, -mu*r) of a
  9-row augmented, pre-scaled x (xs = x*r computed once in the prepass).
  This removes the per-step input projection, LN apply, x0 copy and all
  per-step DMA broadcasts.
- x transposes to feature-major are done 10 timesteps at a time (one PE
  transpose per q-chunk per window) into PSUM; per step a single [9,1024]
  Pool copy produces the matmul rhs.
- Layer 1 lags layer 0 by 2 steps; tanh(c)/h-multiply run one slot after
  their gates, so the ACT engine starts every slot with ready work and is
  the saturated bottleneck: 10 table-ops x [128,1024] per slot.
- bf16 on the h path (h, sig_i, tanh_g, sig_o, tanh_c, u, Whh/Wih1) for DVE
  2x mode and cheap recurrent matmuls; f32 for c, sig_f, v and the x path
  (float32r matmuls).
"""
import sys
sys.path.insert(0, '/opt/trn_rl_repo')

import numpy as np

B, T, F, H, D1, D2, OUT = 8192, 90, 7, 128, 128, 64, 30
NCORES = 8
BL = B // NCORES
G4 = 4 * H
NH = BL // 512
QB = BL // 128
EPS = 1e-5
W = 10            # timesteps per transpose window
NW = T // W       # 9 windows
FA = F + 2        # augmented feature rows: 7 x*r + r + (-mu*r)


LABELS = {}


def _build(nc, T_steps=T, dbg=False):
    LABELS.clear()

    def LBL(ins, label):
        try:
            LABELS[ins.ins.name if hasattr(ins, "ins") else ins.name] = label
        except Exception:
            pass
        return ins

    import concourse.tile as tile
    from concourse import mybir
    from concourse.masks import make_identity

    f32 = mybir.dt.float32
    f32r = mybir.dt.float32r
    bf16 = mybir.dt.bfloat16
    AF = mybir.ActivationFunctionType
    ALU = mybir.AluOpType

    TS = T_steps

    # ---------------- DRAM I/O ----------------
    x_d = nc.dram_tensor("x", [BL, T, F], f32, kind="ExternalInput")
    W_in_d = nc.dram_tensor("W_in", [H, F], f32, kind="ExternalInput")
    b_in_d = nc.dram_tensor("b_in", [H], f32, kind="ExternalInput")
    g_in_d = nc.dram_tensor("g_in", [H], f32, kind="ExternalInput")
    be_in_d = nc.dram_tensor("be_in", [H], f32, kind="ExternalInput")
    Wih_d = [nc.dram_tensor("Wih0", [G4, H], f32, kind="ExternalInput"),
             nc.dram_tensor("Wih1", [G4, H], f32, kind="ExternalInput")]
    Whh_d = [nc.dram_tensor("Whh0", [G4, H], f32, kind="ExternalInput"),
             nc.dram_tensor("Whh1", [G4, H], f32, kind="ExternalInput")]
    bih_d = [nc.dram_tensor("bih0", [G4], f32, kind="ExternalInput"),
             nc.dram_tensor("bih1", [G4], f32, kind="ExternalInput")]
    bhh_d = [nc.dram_tensor("bhh0", [G4], f32, kind="ExternalInput"),
             nc.dram_tensor("bhh1", [G4], f32, kind="ExternalInput")]
    g_ln_d = nc.dram_tensor("g_ln", [H], f32, kind="ExternalInput")
    be_ln_d = nc.dram_tensor("be_ln", [H], f32, kind="ExternalInput")
    W_d1_d = nc.dram_tensor("W_d1", [D1, H], f32, kind="ExternalInput")
    b_d1_d = nc.dram_tensor("b_d1", [D1], f32, kind="ExternalInput")
    W_d2_d = nc.dram_tensor("W_d2", [D2, D1], f32, kind="ExternalInput")
    b_d2_d = nc.dram_tensor("b_d2", [D2], f32, kind="ExternalInput")
    W_d3_d = nc.dram_tensor("W_d3", [OUT, D2], f32, kind="ExternalInput")
    b_d3_d = nc.dram_tensor("b_d3", [OUT], f32, kind="ExternalInput")
    out_d = nc.dram_tensor("out", [BL, OUT], f32, kind="ExternalOutput")
    if dbg:
        dbg_h0 = nc.dram_tensor("dbg_h0", [H, BL], f32, kind="ExternalOutput")
        dbg_h1 = nc.dram_tensor("dbg_h1", [H, BL], f32, kind="ExternalOutput")
        dbg_xs = nc.dram_tensor("dbg_xs", [FA, BL], f32, kind="ExternalOutput")

    import contextlib
    with tile.TileContext(nc) as tc, contextlib.ExitStack() as ctx:
        singles = ctx.enter_context(tc.tile_pool(name="singles", bufs=1))
        big = ctx.enter_context(tc.tile_pool(name="big", bufs=1))
        trans = ctx.enter_context(tc.tile_pool(name="trans", bufs=2))
        scr = ctx.enter_context(tc.tile_pool(name="scr", bufs=1))
        ps_pg = ctx.enter_context(tc.tile_pool(name="ps_pg", bufs=3, space="PSUM"))
        ps_px = ctx.enter_context(tc.tile_pool(name="ps_px", bufs=2, space="PSUM"))
        dpool = ctx.enter_context(tc.tile_pool(name="dpool", bufs=1, space="DRAM"))

        def pg_tile(name):
            return ps_pg.tile([H, BL], f32, tag="pg", name=name)

        def px_small(shape, name):
            return ps_px.tile(shape, f32, tag="px", name=name)

        # ---------------- constants ----------------
        ident = singles.tile([128, 128], f32)
        make_identity(nc, ident)
        ones_row = singles.tile([1, 512], f32)
        nc.vector.memset(ones_row, 1.0)
        ones_col = singles.tile([128, 1], f32)
        nc.vector.memset(ones_col, 1.0)
        eps_col = singles.tile([128, 1], f32)
        nc.vector.memset(eps_col, EPS)

        def load_col(dram_vec, n, name):
            t_ = singles.tile([n, 1], f32, name=name, tag=name)
            nc.sync.dma_start(out=t_, in_=dram_vec[:].rearrange("(p o) -> p o", o=1))
            return t_

        g_in_c = load_col(g_in_d, H, "g_in_c")
        be_in_c = load_col(be_in_d, H, "be_in_c")
        b_in_c = load_col(b_in_d, H, "b_in_c")
        g_ln_c = load_col(g_ln_d, H, "g_ln_c")
        be_ln_c = load_col(be_ln_d, H, "be_ln_c")
        b_d1_c = load_col(b_d1_d, D1, "b_d1_c")
        b_d2_c = load_col(b_d2_d, D2, "b_d2_c")
        b_d3_c = load_col(b_d3_d, OUT, "b_d3_c")

        w_in_raw = singles.tile([H, F], f32)
        nc.sync.dma_start(out=w_in_raw, in_=W_in_d[:, :])

        # ---------------- prepass: LN stats in [T, BL] layout ----------------
        # p' = W_in x + b_in per (h | b,t); over h:
        #   sum p'   = wsum . x + bsum
        #   sum p'^2 = x^T M x + 2 l^T x + c0,  M = W^T W, l = W^T b, c0=|b|^2
        p_m = px_small([F, F], "stat_m")
        nc.tensor.matmul(p_m, w_in_raw, w_in_raw, start=True, stop=True)
        p_ws = px_small([1, F], "stat_ws")
        nc.tensor.matmul(p_ws, ones_col, w_in_raw, start=True, stop=True)
        p_l = px_small([1, F], "stat_l")
        nc.tensor.matmul(p_l, b_in_c, w_in_raw, start=True, stop=True)
        p_sc = px_small([1, 2], "stat_sc")
        nc.tensor.matmul(p_sc[:, 0:1], b_in_c, b_in_c, start=True, stop=False,
                         skip_group_check=True)
        nc.tensor.matmul(p_sc[:, 1:2], ones_col, b_in_c, start=False, stop=True,
                         skip_group_check=True)
        m_sb = trans.tile([F, F], f32, tag="m_sb", name="m_sb")
        nc.vector.tensor_copy(out=m_sb, in_=p_m)
        ws_sb = trans.tile([1, F], f32, tag="ws_sb", name="ws_sb")
        nc.vector.tensor_copy(out=ws_sb, in_=p_ws)
        l_sb = trans.tile([1, F], f32, tag="l_sb", name="l_sb")
        nc.vector.tensor_copy(out=l_sb, in_=p_l)
        sc_sb = trans.tile([1, 2], f32, tag="sc_sb", name="sc_sb")
        nc.vector.tensor_copy(out=sc_sb, in_=p_sc)
        # stage stat constants to DRAM, then partition-broadcast them back
        stat_dram = dpool.tile([F + 2, F * F], f32)
        nc.sync.dma_start(out=stat_dram[0:1, :].rearrange("o (a b) -> (o a) b", a=F),
                          in_=m_sb)
        nc.sync.dma_start(out=stat_dram[F:F + 1, 0:F], in_=ws_sb)
        nc.sync.dma_start(out=stat_dram[F:F + 1, F:2 * F], in_=l_sb)
        nc.sync.dma_start(out=stat_dram[F + 1:F + 2, 0:2], in_=sc_sb)
        wbc = singles.tile([128, F], f32)
        nc.gpsimd.dma_start(out=wbc,
                            in_=stat_dram[F:F + 1, 0:F].to_broadcast([128, F]))
        lbc = singles.tile([128, F], f32)
        nc.gpsimd.dma_start(out=lbc,
                            in_=stat_dram[F:F + 1, F:2 * F].to_broadcast([128, F]))
        scbc = singles.tile([128, 2], f32)
        nc.gpsimd.dma_start(out=scbc,
                            in_=stat_dram[F + 1:F + 2, 0:2].to_broadcast([128, 2]))

        # m2 = 2M - diag(M): coefficients for the upper-triangular quadratic
        m2_sb = trans.tile([F, F], f32, tag="m2_sb", name="m2_sb")
        nc.vector.tensor_tensor(out=m2_sb, in0=m_sb, in1=ident[:F, :F],
                                op=ALU.mult)
        nc.vector.scalar_tensor_tensor(out=m2_sb, in0=m_sb, scalar=2.0,
                                       in1=m2_sb, op0=ALU.mult,
                                       op1=ALU.subtract)
        nc.sync.dma_start(out=stat_dram[1:2, 0:F * F].rearrange(
            "o (a b) -> (o a) b", a=F), in_=m2_sb)
        m2bc = singles.tile([128, F * F], f32)
        nc.gpsimd.dma_start(out=m2bc,
                            in_=stat_dram[1:2, 0:F * F].to_broadcast([128, F * F]))

        # ---------------- x load (row-major, per-q contiguous chunks) --------
        xrow_all = big.tile([128, QB, T, F], f32, tag="xrow", name="xrow_all")
        for q in range(QB):
            nc.sync.dma_start(
                out=xrow_all[:, q],
                in_=x_d[q * 128:(q + 1) * 128, :, :])

        # r_row/nmr_row computed directly in row-major [128, QB, T]
        r_row = singles.tile([128, QB, T], f32)
        nmr_row = singles.tile([128, QB, T], f32)
        nmu_r = singles.tile([128, QB, T], f32)
        QT = [128, QB, T]
        acc_f = scr.tile(QT, f32, tag="st_a", name="st_acc")
        qacc_f = scr.tile(QT, f32, tag="st_b", name="st_qacc")
        yf_f = scr.tile(QT, f32, tag="st_c", name="st_yf")
        yB_f = scr.tile(QT, f32, tag="st_e", name="st_yB")
        tmp_f = scr.tile(QT, f32, tag="st_d", name="st_tmp")
        qaccB_f = scr.tile(QT, f32, tag="st_f", name="st_qaccB")
        linB_f = scr.tile(QT, f32, tag="st_g", name="st_linB")

        def xq(fi):
            return xrow_all[:, :, :TS, fi]

        acc, qacc, yf, tmp = acc_f[:, :, :TS], qacc_f[:, :, :TS], \
            yf_f[:, :, :TS], tmp_f[:, :, :TS]
        qaccB, linB, yB = qaccB_f[:, :, :TS], linB_f[:, :, :TS], yB_f[:, :, :TS]
        # wsum.x on DVE
        nc.vector.tensor_scalar_mul(out=acc, in0=xq(0), scalar1=wbc[:, 0:1])
        for fi in range(1, F):
            nc.vector.scalar_tensor_tensor(
                out=acc, in0=xq(fi), scalar=wbc[:, fi:fi + 1],
                in1=acc, op0=ALU.mult, op1=ALU.add)
        # nmu = -(acc + bsum)/H
        nc.vector.tensor_scalar(out=nmu_r[:, :, :TS], in0=acc,
                                scalar1=scbc[:, 1:2], scalar2=-1.0 / H,
                                op0=ALU.add, op1=ALU.mult)
        # l.x on DVE
        nc.vector.tensor_scalar_mul(out=linB, in0=xq(0), scalar1=lbc[:, 0:1])
        for fi in range(1, F):
            nc.vector.scalar_tensor_tensor(
                out=linB, in0=xq(fi), scalar=lbc[:, fi:fi + 1],
                in1=linB, op0=ALU.mult, op1=ALU.add)
        # upper-tri quadratic: y_i rows on DVE (scalar ops); the x_i*y_i
        # products and the accumulation run on Pool (TensorTensor only).
        ybufs = [yf, yB, tmp, qacc]
        for fi in range(F):
            y_ = ybufs[fi % 4]
            nc.vector.tensor_scalar_mul(
                out=y_, in0=xq(fi),
                scalar1=m2bc[:, fi * F + fi:fi * F + fi + 1])
            for fj in range(fi + 1, F):
                nc.vector.scalar_tensor_tensor(
                    out=y_, in0=xq(fj),
                    scalar=m2bc[:, fi * F + fj:fi * F + fj + 1],
                    in1=y_, op0=ALU.mult, op1=ALU.add)
            if fi == 0:
                nc.gpsimd.tensor_tensor(out=qaccB, in0=xq(fi), in1=y_,
                                        op=ALU.mult)
            else:
                nc.gpsimd.tensor_tensor(out=y_, in0=xq(fi), in1=y_,
                                        op=ALU.mult)
                nc.vector.tensor_add(out=qaccB, in0=qaccB, in1=y_)
        # combine: qacc = qaccB + 2*linB
        nc.vector.scalar_tensor_tensor(out=qacc, in0=linB, scalar=2.0,
                                       in1=qaccB, op0=ALU.mult, op1=ALU.add)
        # var = (q + c0)/H - mu^2 ; r = 1/sqrt(var+eps)
        nc.vector.tensor_scalar(out=qacc, in0=qacc,
                                scalar1=scbc[:, 0:1], scalar2=1.0 / H,
                                op0=ALU.add, op1=ALU.mult)
        nc.vector.tensor_tensor(out=tmp, in0=nmu_r[:, :, :TS],
                                in1=nmu_r[:, :, :TS], op=ALU.mult)
        nc.vector.tensor_sub(out=qacc, in0=qacc, in1=tmp)
        nc.scalar.activation(out=r_row[:, :, :TS], in_=qacc, func=AF.Sqrt,
                             bias=eps_col, scale=1.0)
        nc.vector.reciprocal(out=r_row[:, :, :TS], in_=r_row[:, :, :TS])
        nc.vector.tensor_tensor(out=nmr_row[:, :, :TS], in0=nmu_r[:, :, :TS],
                                in1=r_row[:, :, :TS], op=ALU.mult)

        for q in range(QB):
            nc.sync.dma_start(
                out=xrow_all[:, q],
                in_=x_d[q * 128:(q + 1) * 128, :, :])

        # ---------------- weights: load + PE-transpose ----------------
        def transpose_to(dst, src_ap, p, fdim):
            pt = px_small([fdim, p], "tr_ps")
            nc.tensor.transpose(pt, src_ap, ident[:p, :p])
            if dst.dtype == f32r:
                nc.vector.tensor_copy(out=dst, in_=pt)
            else:
                nc.scalar.copy(out=dst, in_=pt)

        # wihT0g: transposed Wih0 with gamma fold (f32, rhs for Weff build)
        # wihT0f: transposed Wih0 pre-gamma (for bias beta-fold)
        wihT0g = singles.tile([H, 4, H], f32)
        wihT0f = singles.tile([H, 4, H], f32)
        # bf16 recurrent weights
        wihT1 = singles.tile([H, 4, H], bf16, name="wihT1", tag="wihT1")
        whhT = [singles.tile([H, 4, H], bf16, name=f"whhT{L}", tag=f"whhT{L}")
                for L in range(2)]
        for cc in range(4):
            raw = trans.tile([H, H], f32, tag="wraw", name="raw")
            nc.sync.dma_start(out=raw, in_=Wih_d[0][cc * H:(cc + 1) * H, :])
            pt_w = px_small([H, H], "tr_ps")
            nc.tensor.transpose(pt_w, raw, ident)
            nc.scalar.copy(out=wihT0f[:, cc, :], in_=pt_w)
            raw2 = trans.tile([H, H], f32, tag="wraw2", name="raw2")
            nc.sync.dma_start(out=raw2, in_=Wih_d[1][cc * H:(cc + 1) * H, :])
            transpose_to(wihT1[:, cc, :], raw2, H, H)
            for L in range(2):
                raw3 = trans.tile([H, H], f32, tag="wraw", name="raw3")
                nc.sync.dma_start(out=raw3, in_=Whh_d[L][cc * H:(cc + 1) * H, :])
                transpose_to(whhT[L][:, cc, :], raw3, H, H)
        # gamma fold: wihT0g[p, c, m] = Wih0[c*128+m, p] * g_in[p]
        nc.vector.tensor_scalar_mul(
            out=wihT0g[:, :, :].rearrange("p c m -> p (c m)"),
            in0=wihT0f[:, :, :].rearrange("p c m -> p (c m)"),
            scalar1=g_in_c)

        # gate biases beff[L] [128, 4]; layer-0 gets +Wih0 @ be_in (beta fold)
        beff = []
        for L in range(2):
            bt_ = singles.tile([H, 4], f32, name=f"beff{L}", tag=f"beff{L}")
            bih_sb = trans.tile([H, 4], f32, tag="bload", name="bih_sb")
            nc.sync.dma_start(out=bih_sb,
                              in_=bih_d[L][:].rearrange("(c p) -> p c", p=H))
            bhh_sb = trans.tile([H, 4], f32, tag="bload2", name="bhh_sb")
            nc.sync.dma_start(out=bhh_sb,
                              in_=bhh_d[L][:].rearrange("(c p) -> p c", p=H))
            nc.vector.tensor_add(out=bt_, in0=bih_sb, in1=bhh_sb)
            beff.append(bt_)
        for cc in range(4):
            pb = px_small([H, 1], "pb")
            nc.tensor.matmul(pb, wihT0f[:, cc, :], be_in_c, start=True, stop=True)
            nc.vector.tensor_add(out=beff[0][:, cc:cc + 1],
                                 in0=beff[0][:, cc:cc + 1], in1=pb)

        # weff_aug [FA=9, 4, 128] f32 (used via bitcast f32r):
        #  rows 0-6 = (Wih0*diag(g)*W_in)^T ; row 7 = Wih0@(g*b_in); row 8 = Wih0@g
        weff_aug = singles.tile([FA, 4, H], f32r)
        gb_in_c = singles.tile([H, 1], f32)
        nc.vector.tensor_tensor(out=gb_in_c, in0=b_in_c, in1=g_in_c, op=ALU.mult)
        weff_dram = dpool.tile([FA, 4 * H], f32)
        wtmp7 = trans.tile([F, 4 * H], f32, tag="wtmp7", name="wtmp7")
        wtmp1 = trans.tile([1, 4 * H], f32, tag="wtmp1", name="wtmp1")
        wtmp2 = trans.tile([1, 4 * H], f32, tag="wtmp2", name="wtmp2")
        for cc in range(4):
            pwe = px_small([F, H], "pwe")
            nc.tensor.matmul(pwe, w_in_raw, wihT0g[:, cc, :], start=True, stop=True)
            nc.scalar.copy(out=wtmp7[:, cc * H:(cc + 1) * H], in_=pwe)
            pb1 = px_small([1, H], "pb1")
            nc.tensor.matmul(pb1, gb_in_c, wihT0g[:, cc, :], start=True, stop=True)
            nc.scalar.copy(out=wtmp1[:, cc * H:(cc + 1) * H], in_=pb1)
            pb2 = px_small([1, H], "pb2")
            nc.tensor.matmul(pb2, g_in_c, wihT0g[:, cc, :], start=True, stop=True)
            nc.scalar.copy(out=wtmp2[:, cc * H:(cc + 1) * H], in_=pb2)
        nc.sync.dma_start(out=weff_dram[0:F, :], in_=wtmp7)
        nc.sync.dma_start(out=weff_dram[F:F + 1, :], in_=wtmp1)
        nc.sync.dma_start(out=weff_dram[F + 1:F + 2, :], in_=wtmp2)
        weff_stage = trans.tile([FA, 4 * H], f32, tag="weff_stage",
                                name="weff_stage")
        nc.sync.dma_start(out=weff_stage, in_=weff_dram[:, :])
        nc.vector.tensor_copy(
            out=weff_aug[:, :, :].rearrange("p c m -> p (c m)"),
            in_=weff_stage)

        wd1T = singles.tile([H, D1], f32r)
        wd1_raw = trans.tile([D1, H], f32, tag="wraw", name="wd1_raw")
        nc.sync.dma_start(out=wd1_raw, in_=W_d1_d[:, :])
        transpose_to(wd1T, wd1_raw, D1, H)
        wd2T = singles.tile([D1, D2], f32r)
        wd2_raw = trans.tile([D2, D1], f32, tag="wraw2", name="wd2_raw")
        nc.sync.dma_start(out=wd2_raw, in_=W_d2_d[:, :])
        transpose_to(wd2T, wd2_raw, D2, D1)
        wd3T = singles.tile([D2, OUT], f32r)
        wd3_raw = trans.tile([OUT, D2], f32, tag="wraw", name="wd3_raw")
        nc.sync.dma_start(out=wd3_raw, in_=W_d3_d[:, :])
        transpose_to(wd3T, wd3_raw, OUT, D2)

        # xrow_aug[p, q, t, 0:7] = x*r ; [...,7] = r ; [...,8] = nmr
        xrow_aug = big.tile([128, QB, T, FA], f32, tag="xtm", name="xrow_aug")
        for fi in range(F):
            nc.vector.tensor_tensor(
                out=xrow_aug[:, :, :TS, fi],
                in0=xrow_all[:, :, :TS, fi],
                in1=r_row[:, :, :TS], op=ALU.mult)
        nc.scalar.copy(out=xrow_aug[:, :, :TS, F], in_=r_row[:, :, :TS])
        nc.scalar.copy(out=xrow_aug[:, :, :TS, F + 1],
                       in_=nmr_row[:, :, :TS])

        # ---------------- states ----------------
        c = [[singles.tile([H, BL], f32, name=f"c{L}_{i}", tag=f"c{L}_{i}")
              for i in range(2)] for L in range(2)]
        h0_ring = [singles.tile([H, BL], bf16, name=f"h0r{i}", tag=f"h0r{i}")
                   for i in range(2)]
        h1_ring = [singles.tile([H, BL], bf16, name=f"h1r{i}", tag=f"h1r{i}")
                   for i in range(2)]
        for L in range(2):
            for i in range(2):
                nc.vector.memset(c[L][i], 0.0)
        for i in range(2):
            nc.vector.memset(h0_ring[i], 0.0)
            nc.vector.memset(h1_ring[i], 0.0)
        h1_final = singles.tile([H, BL], bf16, name="h1fin", tag="h1fin")

        # ---------------- per-step transposes + xs copies ----------------
        xs_tiles = {}

        def emit_xs_copy(t):
            """PE-transpose step t to feature-major, then DVE copies to SBUF."""
            xst = trans.tile([FA, BL], f32r, tag="xs", name=f"xs{t}")
            for half in range(2):
                pxt = ps_px.tile([FA, 512], f32, tag="px", name=f"px{t}_{half}")
                for qi in range(4):
                    q = half * 4 + qi
                    nc.tensor.transpose(pxt[:, qi * 128:(qi + 1) * 128],
                                        xrow_aug[:, q, t, :], ident)
                LBL(nc.vector.tensor_copy(
                    out=xst[:, half * 512:(half + 1) * 512], in_=pxt),
                    f"xscp{half}")
            xs_tiles[t] = xst

        emit_xs_copy(0)

        # ---------------- main loop ----------------
        # slot t: A: tanh/h for L0 step t-1 (shifted); C: L0 gates step t;
        #         D: L1 gates step t-2 with tanh/h inline at slot end.
        # ACT order/slot: tc0, tg1, si1, tg0, sf1, si0, sf0, so1, tc1, so0
        # DVE order/slot: h0, u1, v1, c1, u0, v0, c0, h1
        so_prev0 = None

        def R(ap):
            return ap.bitcast(f32r)

        n_slots = TS + 2
        for t in range(n_slots):
            do_A = 1 <= t <= TS
            do_C = t <= TS - 1
            do_D = 2 <= t <= TS + 1

            # --- A: h0_{t-1} = so0_prev * tanh(c0_{t-1}) ---
            if do_A:
                tc0 = scr.tile([H, BL], bf16, tag="tc0", name="tc0")
                LBL(nc.scalar.activation(out=tc0, in_=c[0][(t - 1) % 2],
                                         func=AF.Tanh, scale=1.0), "tc0")
                LBL(nc.vector.tensor_tensor(out=h0_ring[t % 2], in0=so_prev0,
                                            in1=tc0, op=ALU.mult), "h0")

            # --- matmuls, interleaved D/C, gate order matched to ACT order ---
            pg1 = {}
            pg0 = {}

            def emit_mm_D(gc):
                pg = pg_tile(f"pg1_{gc}")
                h0_in = h0_ring[(t - 1) % 2]
                h1_in = h1_ring[(t - 1) % 2]
                for hc in range(NH):
                    sl = slice(hc * 512, (hc + 1) * 512)
                    nc.tensor.matmul(pg[:, sl], wihT1[:, gc, :], h0_in[:, sl],
                                     start=True, stop=False,
                                     skip_group_check=True)
                    nc.tensor.matmul(pg[:, sl], whhT[1][:, gc, :], h1_in[:, sl],
                                     start=False, stop=True,
                                     skip_group_check=True)
                pg1[gc] = pg

            def emit_mm_C(gc, xst):
                pg = pg_tile(f"pg0_{gc}")
                h0_rec = h0_ring[t % 2]
                for hc in range(NH):
                    sl = slice(hc * 512, (hc + 1) * 512)
                    nc.tensor.matmul(pg[:, sl], weff_aug[:, gc, :],
                                     xst[:, sl],
                                     start=True, stop=False,
                                     skip_group_check=True)
                    nc.tensor.matmul(pg[:, sl], whhT[0][:, gc, :],
                                     h0_rec[:, sl],
                                     start=False, stop=True,
                                     skip_group_check=True)
                pg0[gc] = pg

            xst = xs_tiles.pop(t) if do_C else None
            # mm emission: D [g1,g0,g2], C [g2], D [g3], C [g0,g1,g3]
            if do_D:
                emit_mm_D(1)
                emit_mm_D(0)
                emit_mm_D(2)
            if do_C:
                emit_mm_C(2, xst)
            if do_D:
                emit_mm_D(3)
            if do_C:
                emit_mm_C(0, xst)
                emit_mm_C(1, xst)
                emit_mm_C(3, xst)

            def act_gate(pg, L, gc, dt_):
                funcs = {0: AF.Sigmoid, 1: AF.Sigmoid, 2: AF.Tanh, 3: AF.Sigmoid}
                o = scr.tile([H, BL], dt_, tag=f"g{L}_{gc}", name=f"g{L}_{gc}")
                LBL(nc.scalar.activation(out=o, in_=pg, func=funcs[gc],
                                         bias=beff[L][:, gc:gc + 1], scale=1.0),
                    f"g{L}_{gc}")
                return o

            # ACT: sf1, si1, tg1 | tg0 ; DVE: v1, u1, c1
            if do_D:
                sf1 = act_gate(pg1[1], 1, 1, f32)
                si1 = act_gate(pg1[0], 1, 0, bf16)
                tg1 = act_gate(pg1[2], 1, 2, bf16)
            if do_C:
                tg0 = act_gate(pg0[2], 0, 2, bf16)
            if do_D:
                v1 = scr.tile([H, BL], f32, tag="v1", name="v1")
                LBL(nc.vector.tensor_tensor(out=v1, in0=sf1,
                                            in1=c[1][(t - 1) % 2],
                                            op=ALU.mult), "v1")
                u1 = scr.tile([H, BL], bf16, tag="u1", name="u1")
                LBL(nc.vector.tensor_tensor(out=u1, in0=si1, in1=tg1,
                                            op=ALU.mult), "u1")
                LBL(nc.vector.tensor_add(out=c[1][t % 2], in0=u1, in1=v1), "c1")
            # ACT: si0, sf0, so1 ; DVE: u0, v0
            if do_C:
                si0 = act_gate(pg0[0], 0, 0, bf16)
                sf0 = act_gate(pg0[1], 0, 1, f32)
            if do_D:
                so1 = act_gate(pg1[3], 1, 3, bf16)
            if do_C:
                u0 = scr.tile([H, BL], bf16, tag="u0", name="u0")
                LBL(nc.vector.tensor_tensor(out=u0, in0=si0, in1=tg0,
                                            op=ALU.mult), "u0")
                v0 = scr.tile([H, BL], f32, tag="v0", name="v0")
                LBL(nc.vector.tensor_tensor(out=v0, in0=sf0,
                                            in1=c[0][(t - 1) % 2],
                                            op=ALU.mult), "v0")
            # ACT: tc1 ; DVE: c0, h1 ; ACT: so0
            if do_D:
                tc1 = scr.tile([H, BL], bf16, tag="tc1", name="tc1")
                LBL(nc.scalar.activation(out=tc1, in_=c[1][t % 2], func=AF.Tanh,
                                         scale=1.0), "tc1")
            if do_C:
                LBL(nc.vector.tensor_add(out=c[0][t % 2], in0=u0, in1=v0), "c0")
            if do_D:
                LBL(nc.vector.tensor_tensor(out=h1_ring[t % 2], in0=so1,
                                              in1=tc1, op=ALU.mult), "h1")
                if t == TS + 1:
                    nc.vector.tensor_copy(out=h1_final, in_=h1_ring[t % 2])
            if do_C:
                so0 = trans.tile([H, BL], bf16, tag="so0", name="so0")
                LBL(nc.scalar.activation(out=so0, in_=pg0[3], func=AF.Sigmoid,
                                         bias=beff[0][:, 3:4], scale=1.0), "so0")
                so_prev0 = so0

            # xs prefetch for next slot (PE transposes after the slot's mms)
            if t + 1 <= TS - 1:
                emit_xs_copy(t + 1)

        # ---------------- head ----------------
        h1f = scr.tile([H, BL], f32r, tag="st_a", name="h1f")
        nc.vector.tensor_copy(out=h1f, in_=h1_final)
        sqh = scr.tile([H, BL], f32r, tag="st_b", name="sqh")
        nc.vector.tensor_tensor(out=sqh, in0=h1f, in1=h1f, op=ALU.mult)
        ones_col_r = singles.tile([128, 1], f32r)
        nc.vector.tensor_copy(out=ones_col_r, in_=ones_col)
        ones_row_r = singles.tile([1, 128], f32r)
        nc.vector.tensor_copy(out=ones_row_r, in_=ones_row[0:1, 0:128])
        ps_s1 = ps_pg.tile([1, BL], f32, tag="pg", name="ps_s1")
        ps_s2 = ps_pg.tile([1, BL], f32, tag="pg", name="ps_s2")
        for hc in range(NH):
            sl = slice(hc * 512, (hc + 1) * 512)
            nc.tensor.matmul(ps_s1[:, sl], ones_col_r, h1f[:, sl],
                             start=True, stop=True, skip_group_check=True)
            nc.tensor.matmul(ps_s2[:, sl], ones_col_r, sqh[:, sl],
                             start=True, stop=True, skip_group_check=True)
        nmu_h = singles.tile([1, BL], f32r, tag="nmu_h", name="nmu_h")
        nc.vector.tensor_scalar_mul(out=nmu_h, in0=ps_s1, scalar1=-1.0 / H)
        musq_h = singles.tile([1, BL], f32, tag="musq", name="musq_h")
        nc.vector.tensor_tensor(out=musq_h, in0=nmu_h, in1=nmu_h, op=ALU.mult)
        v_h = singles.tile([1, BL], f32, tag="v_h", name="v_h")
        nc.vector.tensor_scalar_mul(out=v_h, in0=ps_s2, scalar1=1.0 / H)
        nc.vector.tensor_sub(out=v_h, in0=v_h, in1=musq_h)
        nc.scalar.activation(out=v_h, in_=v_h, func=AF.Sqrt,
                             bias=eps_col[0:1], scale=1.0)
        v_hr = singles.tile([1, BL], f32r, tag="v_hr", name="v_hr")
        with nc.allow_low_precision(reason="f32r rounding for PE broadcast"):
            nc.vector.reciprocal(out=v_hr, in_=v_h)
        # broadcast LN stats across partitions via PE outer products
        pnm = ps_pg.tile([H, BL], f32, tag="pg", name="pnm")
        prh = ps_pg.tile([H, BL], f32, tag="pg", name="prh")
        for hc in range(NH):
            sl = slice(hc * 512, (hc + 1) * 512)
            nc.tensor.matmul(pnm[:, sl], ones_row_r, nmu_h[:, sl],
                             start=True, stop=True, skip_group_check=True)
            nc.tensor.matmul(prh[:, sl], ones_row_r, v_hr[:, sl],
                             start=True, stop=True, skip_group_check=True)
        t1 = scr.tile([H, BL], f32, tag="st_c", name="t1")
        nc.vector.tensor_tensor(out=t1, in0=h1f, in1=pnm, op=ALU.add)
        t2 = scr.tile([H, BL], f32, tag="st_d", name="t2")
        nc.vector.tensor_tensor(out=t2, in0=t1, in1=prh, op=ALU.mult)
        last = scr.tile([H, BL], f32r, tag="st_e", name="last")
        nc.vector.tensor_scalar(out=last, in0=t2, scalar1=g_ln_c,
                                scalar2=be_ln_c, op0=ALU.mult, op1=ALU.add)
        pd1 = pg_tile("pd1")
        for hc in range(NH):
            sl = slice(hc * 512, (hc + 1) * 512)
            nc.tensor.matmul(pd1[:D1, sl], wd1T, last[:, sl], start=True,
                             stop=True, skip_group_check=True)
        d1 = scr.tile([D1, BL], f32r, tag="st_a", name="d1")
        nc.scalar.activation(out=d1, in_=pd1[:D1], func=AF.Relu, bias=b_d1_c,
                             scale=1.0)
        pd2 = pg_tile("pd2")
        for hc in range(NH):
            sl = slice(hc * 512, (hc + 1) * 512)
            nc.tensor.matmul(pd2[:D2, sl], wd2T, d1[:, sl], start=True,
                             stop=True, skip_group_check=True)
        d2 = scr.tile([D2, BL], f32r, tag="st_b", name="d2")
        nc.scalar.activation(out=d2, in_=pd2[:D2], func=AF.Relu, bias=b_d2_c,
                             scale=1.0)
        pd3 = pg_tile("pd3")
        for hc in range(NH):
            sl = slice(hc * 512, (hc + 1) * 512)
            nc.tensor.matmul(pd3[:OUT, sl], wd3T, d2[:, sl], start=True,
                             stop=True, skip_group_check=True)
        o3 = scr.tile([OUT, BL], f32, tag="st_c", name="o3")
        nc.scalar.activation(out=o3, in_=pd3[:OUT], func=AF.Identity,
                             bias=b_d3_c, scale=1.0)
        outT = singles.tile([128, QB, OUT], f32)
        for q in range(QB):
            pot = px_small([128, OUT], "pot")
            nc.tensor.transpose(pot, o3[:, q * 128:(q + 1) * 128],
                                ident[:OUT, :OUT])
            nc.vector.tensor_copy(out=outT[:, q, :], in_=pot)
        nc.sync.dma_start(
            out=out_d[:, :].rearrange("(q p) c -> p q c", p=128),
            in_=outT)
        if dbg:
            h0f = scr.tile([H, BL], f32, tag="st_a", name="h0f")
            nc.vector.tensor_copy(out=h0f, in_=h0_ring[(TS) % 2])
            nc.sync.dma_start(out=dbg_h0[:, :], in_=h0f)
            nc.sync.dma_start(out=dbg_h1[:, :], in_=h1f)
    return nc


_CACHE = {}


def _get_runner():
    if "runner" in _CACHE:
        return _CACHE["runner"]
    import jax
    from jax.sharding import Mesh, PartitionSpec
    from jax.experimental.shard_map import shard_map
    import concourse.bacc as bacc
    import concourse.mybir as mybir
    from concourse.bass2jax import install_neuronx_cc_hook, _bass_exec_p, \
        partition_id_tensor

    nc = bacc.Bacc()
    _build(nc)
    nc.compile()
    install_neuronx_cc_hook()

    partition_name = nc.partition_id_tensor.name if nc.partition_id_tensor else None
    in_names, out_names, out_avals, zero_outs = [], [], [], []
    for alloc in nc.m.functions[0].allocations:
        if not isinstance(alloc, mybir.MemoryLocationSet):
            continue
        name = alloc.memorylocations[0].name
        if alloc.kind == "ExternalInput":
            if name != partition_name:
                in_names.append(name)
        elif alloc.kind == "ExternalOutput":
            out_names.append(name)
            shape = tuple(alloc.tensor_shape)
            dtype = mybir.dt.np(alloc.dtype)
            out_avals.append(jax.core.ShapedArray(shape, dtype))
            zero_outs.append(np.zeros(shape, dtype))
    n_params = len(in_names)
    all_in_names = in_names + out_names + ([partition_name] if partition_name else [])

    def _body(*args):
        operands = list(args)
        if partition_name is not None:
            operands.append(partition_id_tensor())
        outs = _bass_exec_p.bind(
            *operands,
            out_avals=tuple(out_avals),
            in_names=tuple(all_in_names),
            out_names=tuple(out_names),
            lowering_input_output_aliases=(),
            sim_require_finite=False,
            sim_require_nnan=False,
            nc=nc,
        )
        return tuple(outs)

    devices = jax.devices()[:NCORES]
    mesh = Mesh(np.asarray(devices), ("core",))
    in_specs = (PartitionSpec("core"),) * (n_params + len(out_names))
    out_specs = (PartitionSpec("core"),) * len(out_names)
    sharded = jax.jit(
        shard_map(_body, mesh=mesh, in_specs=in_specs, out_specs=out_specs,
                  check_rep=False),
        keep_unused=True)
    _CACHE["runner"] = (sharded, in_names, out_names, zero_outs)
    return _CACHE["runner"]


def kernel(**inputs) -> np.ndarray:
    sharded, in_names, out_names, zero_outs = _get_runner()
    inp = {k: np.ascontiguousarray(np.asarray(v), dtype=np.float32)
           for k, v in inputs.items()}

    def core_val(name, ci):
        if name == "x":
            return inp["x"][ci * BL:(ci + 1) * BL]
        return inp[name]

    concat_in = [
        np.concatenate([core_val(n, ci) for ci in range(NCORES)], axis=0)
        for n in in_names
    ]
    concat_zeros = [
        np.zeros((NCORES * z.shape[0], *z.shape[1:]), z.dtype) for z in zero_outs
    ]
    import jax
    out_arrs = sharded(*concat_in, *concat_zeros)
    jax.block_until_ready(out_arrs)
    oi = out_names.index("out")
    full = np.asarray(out_arrs[oi]).reshape(B, OUT)
    return full.astype(np.float32)
